# revision 1
# baseline (speedup 1.0000x reference)
"""ODE-RNN Trainium2 kernel.

Problem: out[b, t*8+i, :] = 2-layer GRU (H=1024) run over the batch dim
(64 steps) of sequence t (30 sequences), with initial hiddens taken from an
RK4-integrated ODE trajectory (8 grid points, shared across all runs).

Strategy (8 NeuronCores, pure data-parallel, no collectives):
  - The ODE trajectory (128 sequential tiny (2,1024) MLP evals, <1.2% of
    FLOPs, latency-serial and weight-streaming-bound on a systolic array)
    is computed on the host in fp32, exactly mirroring the reference math.
  - Core i handles the 30 GRU runs with init traj[i] (data-parallel over the
    240 independent (t,i) runs; weights replicated per core).
  - Per core, the GRU is restructured into 4 phases:
      A: gi1 = X @ wi0.T + bias  (dense, M=2048)             -> DRAM
      B: layer-1 recurrence, 64 steps, state batched M=32    -> h1 states
      C: gi2 = H1states @ wi1.T + bias (dense, M=2048)       -> DRAM
      D: layer-2 recurrence, 64 steps                        -> output
    The recurrent matmuls keep the state transposed ([H,parts] x runs) as the
    PE stationary operand and stream the (resident) recurrent weights as the
    moving operand; the state transpose is maintained with PE transposes.
  - All matmuls run in float32r (1 cycle/row, ~1.5e-4 rel err measured).
"""

import numpy as np

try:
    import concourse.bass as bass  # noqa: F401
except ImportError:  # pragma: no cover - fallback for bare environments
    import sys
    sys.path.insert(0, "/opt/trn_rl_repo")
    import concourse.bass as bass  # noqa: F401

import concourse.mybir as mybir
import concourse.tile as tile
from concourse import bacc
from concourse.bass_utils import run_bass_kernel_spmd
from concourse.masks import make_identity

F32 = mybir.dt.float32
F32R = mybir.dt.float32r
AF = mybir.ActivationFunctionType

H = 1024        # hidden size
G3 = 3 * H      # gate width
KC = H // 128   # K chunks
T = 30          # sequences
R = 32          # padded runs per core (30 real + 2 pad)
NSEG = 8
SUB = 4
NCORES = 8


def build_nc(steps=64):
    """Build the per-core Bass module (same program on all 8 cores)."""
    MT = steps * R            # gi row count (2048 for steps=64)
    MCH = MT // 128           # M chunks (16)
    nc = bacc.Bacc()

    xtr = nc.declare_dram_parameter("xtr", [128, KC, MT], F32R, isOutput=False)
    wi0t = nc.declare_dram_parameter("wi0t", [H, G3], F32R, isOutput=False)
    wh0t = nc.declare_dram_parameter("wh0t", [H, G3], F32R, isOutput=False)
    wi1t = nc.declare_dram_parameter("wi1t", [H, G3], F32R, isOutput=False)
    wh1t = nc.declare_dram_parameter("wh1t", [H, G3], F32R, isOutput=False)
    bias1 = nc.declare_dram_parameter("bias1", [G3], F32, isOutput=False)
    bias2 = nc.declare_dram_parameter("bias2", [G3], F32, isOutput=False)
    bhn1 = nc.declare_dram_parameter("bhn1", [H], F32, isOutput=False)
    bhn2 = nc.declare_dram_parameter("bhn2", [H], F32, isOutput=False)
    h1f0 = nc.declare_dram_parameter("h1f0", [R, H], F32, isOutput=False)
    h2f0 = nc.declare_dram_parameter("h2f0", [R, H], F32, isOutput=False)
    h1t0 = nc.declare_dram_parameter("h1t0", [128, KC, R], F32R, isOutput=False)
    h2t0 = nc.declare_dram_parameter("h2t0", [128, KC, R], F32R, isOutput=False)
    out = nc.declare_dram_parameter("out", [steps, R, H], F32, isOutput=True)

    gi1b = nc.dram_tensor("gi1b", [MT, G3], F32R)
    gi2b = nc.dram_tensor("gi2b", [MT, G3], F32R)
    h1ts = nc.dram_tensor("h1ts", [128, KC, steps, R], F32R)

    def bcast(ap, parts=128):
        return bass.AP(tensor=ap.tensor, offset=ap.offset,
                       ap=[[0, parts]] + list(ap.ap))

    with tile.TileContext(nc) as tc:
        with (
            tc.tile_pool(name="wp", bufs=KC) as wp,
            tc.tile_pool(name="const", bufs=1) as const,
        ):
            # --- constants ---
            bias1_bc = const.tile([128, G3], F32)
            nc.sync.dma_start(out=bias1_bc, in_=bcast(bias1[:]))
            bias2_bc = const.tile([128, G3], F32)
            nc.sync.dma_start(out=bias2_bc, in_=bcast(bias2[:]))
            bhn1_bc = const.tile([R, H], F32)
            nc.sync.dma_start(out=bhn1_bc, in_=bcast(bhn1[:], parts=R))
            bhn2_bc = const.tile([R, H], F32)
            nc.sync.dma_start(out=bhn2_bc, in_=bcast(bhn2[:], parts=R))
            ident_g = const.tile([32, 32], F32)
            make_identity(nc, ident_g)
            ident = const.tile([32, 32], F32)
            nc.vector.tensor_copy(ident, ident_g)
            ident_r = const.tile([32, 32], F32R)
            nc.vector.tensor_copy(ident_r, ident_g)

            # --- state tiles ---
            h1f = const.tile([R, H], F32)
            nc.sync.dma_start(out=h1f, in_=h1f0[:])
            h2f = const.tile([R, H], F32)
            nc.sync.dma_start(out=h2f, in_=h2f0[:])
            h1t = const.tile([128, KC, R], F32R)
            nc.sync.dma_start(out=h1t, in_=h1t0[:])
            h2t = const.tile([128, KC, R], F32R)
            nc.sync.dma_start(out=h2t, in_=h2t0[:])

            def load_weight(param, label):
                tiles = []
                for k in range(KC):
                    wt = wp.tile([128, G3], F32R, tag="w", name=f"w_{label}_{k}")
                    nc.sync.dma_start(out=wt, in_=param[k * 128:(k + 1) * 128, :])
                    tiles.append(wt)
                return tiles

            def phase_gi(wtiles, bias_bc, gib, lhs_loader, nm):
                """gi = lhsT.T @ W + bias for MCH M-chunks of 128 rows."""
                with (
                    tc.tile_pool(name=f"psA{nm}", bufs=2, space="PSUM") as psA,
                    tc.tile_pool(name=f"evp{nm}", bufs=2) as evp,
                    tc.tile_pool(name=f"lhsp{nm}", bufs=2) as lhsp,
                ):
                    lhs = None
                    for m in range(MCH):
                        lhs, msub = lhs_loader(lhsp, m, lhs)
                        for half in range(2):
                            ps = psA.tile([128, 1536], F32, tag="ps", name=f"ps{nm}_{m}_{half}")
                            for n3 in range(3):
                                ncol = half * 1536 + n3 * 512
                                for k in range(KC):
                                    nc.tensor.matmul(
                                        ps[:, n3 * 512:(n3 + 1) * 512],
                                        lhs[:, k, msub * 128:(msub + 1) * 128],
                                        wtiles[k][:, ncol:ncol + 512],
                                        start=(k == 0), stop=(k == KC - 1))
                            ev = evp.tile([128, 1536], F32R, tag="ev", name=f"ev{nm}_{m}_{half}")
                            nc.vector.tensor_add(
                                ev, ps, bias_bc[:, half * 1536:(half + 1) * 1536])
                            nc.sync.dma_start(
                                out=gib[m * 128:(m + 1) * 128,
                                        half * 1536:(half + 1) * 1536],
                                in_=ev)

            def lhs_loader_A(lhsp, m, lhs):
                # xtr chunks: up to 4 M-chunks per DMA ([128, KC, <=512] tiles)
                if m % 4 == 0:
                    width = min(512, (MCH - m) * 128)
                    lhs = lhsp.tile([128, KC, width], F32R, tag="lhsA", name=f"lhsA_{m}")
                    nc.sync.dma_start(
                        out=lhs, in_=xtr[:, :, m * 128:m * 128 + width])
                return lhs, m % 4

            def lhs_loader_C(lhsp, m, lhs):
                # h1ts slice: steps 4m..4m+4 -> [128, KC, 128] (s-major, t-minor)
                lhs = lhsp.tile([128, KC, 4, R], F32R, tag="lhsC", name=f"lhsC_{m}")
                nc.sync.dma_start(out=lhs, in_=h1ts[:, :, 4 * m:4 * m + 4, :])
                return lhs.rearrange("p k s t -> p k (s t)"), 0

            def phase_rec(wtiles, gib, bhn_bc, hf, ht, save, outd, nm):
                with (
                    tc.tile_pool(name=f"ghp{nm}", bufs=7, space="PSUM") as ghp,
                    tc.tile_pool(name=f"trp{nm}", bufs=1, space="PSUM") as trpp,
                    tc.tile_pool(name=f"gp{nm}", bufs=14) as gp,
                    tc.tile_pool(name=f"gip{nm}", bufs=2) as gip,
                ):
                    for s in range(steps):
                        gi = gip.tile([R, G3], F32R, tag="gi", name=f"gi{nm}_{s}")
                        nc.sync.dma_start(out=gi, in_=gib[s * R:(s + 1) * R, :])
                        # K-split accumulation: the k<4 half depends only on
                        # ht chunks 0-3 (rewritten by the previous step's
                        # first gate slice), so it can overlap the previous
                        # step's second-slice gates instead of waiting for
                        # the full state update.
                        ghs = {}
                        for kh in range(2):
                            for n in (0, 2, 4, 1, 3, 5):
                                if kh == 0:
                                    ghs[n] = ghp.tile([R, 512], F32, tag="gh",
                                                      name=f"gh{nm}_{s}_{n}")
                                gh = ghs[n]
                                for k in range(kh * 4, kh * 4 + 4):
                                    nc.tensor.matmul(
                                        gh, ht[:, k, :],
                                        wtiles[k][:, n * 512:(n + 1) * 512],
                                        start=(k == 0),
                                        stop=(k == KC - 1 and n >= 4))
                                if kh == 1 and n < 4:
                                    # r/z gates: accumulate gi (incl. biases)
                                    # on the PE so ACT can sigmoid PSUM
                                    # directly (saves 2 DVE adds per slice).
                                    nc.tensor.matmul(
                                        gh, ident_r, gi[:, n * 512:(n + 1) * 512],
                                        start=False, stop=True)
                        trp = trpp.tile([128, KC, R], F32, tag="tr", name=f"tr{nm}_{s}")
                        for j in range(2):
                            c0 = j * 512
                            t = lambda nmm: gp.tile([R, 512], F32, tag="gt", name=f"{nmm}{nm}_{s}_{j}")
                            rr = t("rr")
                            nc.scalar.activation(rr, ghs[j], AF.Sigmoid)
                            zz = t("zz")
                            nc.scalar.activation(zz, ghs[2 + j], AF.Sigmoid)
                            hn = t("hn")
                            nc.vector.tensor_add(hn, ghs[4 + j], bhn_bc[:, c0:c0 + 512])
                            t1 = t("t1")
                            nc.vector.tensor_mul(t1, rr, hn)
                            npre = t("npre")
                            nc.vector.tensor_add(npre, t1, gi[:, 2 * H + c0:2 * H + c0 + 512])
                            nn = t("nn")
                            nc.scalar.activation(nn, npre, AF.Tanh)
                            dd = t("dd")
                            nc.vector.tensor_sub(dd, hf[:, c0:c0 + 512], nn)
                            t2 = t("t2")
                            nc.vector.tensor_mul(t2, zz, dd)
                            nc.vector.tensor_add(hf[:, c0:c0 + 512], nn, t2)
                            for c in range(j * 4, j * 4 + 4):
                                nc.tensor.transpose(
                                    trp[:, c, :], hf[:, c * 128:(c + 1) * 128], ident)
                            for c in range(j * 4, j * 4 + 4):
                                nc.vector.tensor_copy(ht[:, c, :], trp[:, c, :])
                        if save is not None:
                            nc.sync.dma_start(out=save[:, :, s, :], in_=ht)
                        if outd is not None:
                            nc.sync.dma_start(out=outd[s], in_=hf)

            w = load_weight(wi0t, "i0")
            phase_gi(w, bias1_bc, gi1b, lhs_loader_A, "A")
            w = load_weight(wh0t, "h0")
            phase_rec(w, gi1b, bhn1_bc, h1f, h1t, h1ts, None, "B")
            w = load_weight(wi1t, "i1")
            phase_gi(w, bias2_bc, gi2b, lhs_loader_C, "C")
            w = load_weight(wh1t, "h1")
            phase_rec(w, gi2b, bhn2_bc, h2f, h2t, None, out, "D")

    nc.finalize()
    return nc


def ode_traj(w1, b1, w2, b2, w3, b3):
    """RK4 trajectory of the ODE, mirroring the reference exactly (fp32)."""
    w1t = w1.T.astype(np.float32)
    w2t = w2.T.astype(np.float32)
    w3t = w3.T.astype(np.float32)

    def f(h):
        a = np.tanh(h @ w1t + b1)
        a = np.tanh(a @ w2t + b2)
        return a @ w3t + b3

    dt = np.float32((1.0 / NSEG) / SUB)
    h = np.zeros((2, H), np.float32)
    traj = []
    for _ in range(NSEG):
        for _ in range(SUB):
            k1 = f(h)
            k2 = f(h + np.float32(0.5) * dt * k1)
            k3 = f(h + np.float32(0.5) * dt * k2)
            k4 = f(h + dt * k3)
            h = h + (dt / np.float32(6.0)) * (k1 + np.float32(2.0) * k2
                                              + np.float32(2.0) * k3 + k4)
        traj.append(h.copy())
    return np.stack(traj)  # (NSEG, 2, H)


def make_in_maps(x, w1, b1, w2, b2, w3, b3, wi0, wh0, bi0, bh0,
                 wi1, wh1, bi1, bh1, steps=64, cores=NCORES):
    traj = ode_traj(w1, b1, w2, b2, w3, b3)
    MT = steps * R

    # xtr[p, k, s*R + t] = x[s, t, k*128+p]
    xp = np.zeros((steps, R, H), np.float32)
    xp[:, :T, :] = x[:steps, :, :]
    xtr = np.ascontiguousarray(
        xp.reshape(MT, KC, 128).transpose(2, 1, 0))

    shared = {
        "xtr": xtr,
        "wi0t": np.ascontiguousarray(wi0.T),
        "wh0t": np.ascontiguousarray(wh0.T),
        "wi1t": np.ascontiguousarray(wi1.T),
        "wh1t": np.ascontiguousarray(wh1.T),
        "bias1": np.concatenate([bi0[:2 * H] + bh0[:2 * H], bi0[2 * H:]]),
        "bias2": np.concatenate([bi1[:2 * H] + bh1[:2 * H], bi1[2 * H:]]),
        "bhn1": np.ascontiguousarray(bh0[2 * H:]),
        "bhn2": np.ascontiguousarray(bh1[2 * H:]),
    }
    in_maps = []
    for i in range(cores):
        h1 = traj[i, 0]
        h2 = traj[i, 1]
        m = dict(shared)
        m["h1f0"] = np.ascontiguousarray(np.tile(h1, (R, 1)))
        m["h2f0"] = np.ascontiguousarray(np.tile(h2, (R, 1)))
        m["h1t0"] = np.ascontiguousarray(
            np.broadcast_to(h1.reshape(KC, 128).T[:, :, None], (128, KC, R)))
        m["h2t0"] = np.ascontiguousarray(
            np.broadcast_to(h2.reshape(KC, 128).T[:, :, None], (128, KC, R)))
        in_maps.append(m)
    return in_maps


_NC_CACHE = {}


def _get_nc(steps):
    if steps not in _NC_CACHE:
        _NC_CACHE[steps] = build_nc(steps)
    return _NC_CACHE[steps]


def run_cores(inputs, steps=64, cores=NCORES, **run_kwargs):
    in_maps = make_in_maps(steps=steps, cores=cores, **inputs)
    nc = _get_nc(steps)
    return run_bass_kernel_spmd(nc, in_maps, core_ids=list(range(cores)),
                                **run_kwargs)


def kernel(x, w1, b1, w2, b2, w3, b3, wi0, wh0, bi0, bh0,
           wi1, wh1, bi1, bh1):
    x = np.asarray(x, np.float32)
    args = dict(x=x, w1=w1, b1=b1, w2=w2, b2=b2, w3=w3, b3=b3,
                wi0=wi0, wh0=wh0, bi0=bi0, bh0=bh0,
                wi1=wi1, wh1=wh1, bi1=bi1, bh1=bh1)
    args = {k: np.asarray(v, np.float32) for k, v in args.items()}
    res = run_cores(args, steps=64, cores=NCORES)
    B = 64
    full = np.empty((B, T * NCORES, H), np.float32)
    for i in range(NCORES):
        full[:, i::NCORES, :] = res.results[i]["out"][:, :T, :]
    return full



# revision 13
# speedup vs baseline: 2.9566x; 2.9566x over previous
"""ODE-RNN Trainium2 kernel (v2: feature-major bf16 formulation).

Problem: out[b, t*8+i, :] = 2-layer GRU (H=1024) run over the batch dim
(64 steps) of sequence t (30 sequences), with initial hiddens taken from an
RK4-integrated ODE trajectory (8 grid points, shared across all runs).

Strategy (8 NeuronCores, pure data-parallel, no collectives):
  - ODE trajectory on host (tiny, exactly mirrors reference math).
  - Core i handles the 30 GRU runs with init traj[i]; R=32 padded runs.
  - Everything on-device is kept FEATURE-MAJOR (transposed): state tensors
    live as [128 partitions = feature-within-chunk, KC=8 chunks, R runs].
    The recurrent matmuls then use the (resident, bf16) weights as the
    128x128 PE-stationary operand and stream the small [128, R] state as
    the moving operand: cost-model rows per step drop 4x vs streaming the
    weights, and no PE transposes are needed anywhere.
  - Phase A: gi1 = wi0 @ x.T + bias as a dense GEMM, output to DRAM in a
    per-step-sliceable layout (bf16).
  - Fused step loop (phases B+C+D): per step s, layer-1 gates from
    (gi1[s], h1[s-1]); gi2 = wi1 @ h1[s] accumulated directly into the
    layer-2 gate PSUM; layer-2 gates -> h2[s] -> output staging buffer.
    All biases are accumulated on the PE (identity / ones-row matmuls),
    so the vector/scalar engines only do the nonlinear gate math.
  - Emission is ordered so the PE never waits on the gate-math chains:
    layer-1 gate math overlaps layer-2 recurrent matmuls, and the fresh
    states are consumed k-half by k-half.
"""

import numpy as np

try:
    import concourse.bass as bass  # noqa: F401
except ImportError:  # pragma: no cover - fallback for bare environments
    import sys
    sys.path.insert(0, "/opt/trn_rl_repo")
    import concourse.bass as bass  # noqa: F401

import concourse.mybir as mybir
import concourse.tile as tile
from concourse import bacc
from concourse.bass_utils import run_bass_kernel_spmd
from concourse.masks import make_identity

F32 = mybir.dt.float32
BF16 = mybir.dt.float16  # fp16: same PE cost as bf16, more mantissa
NP_BF16 = mybir.dt.np(mybir.dt.float16)
AF = mybir.ActivationFunctionType

H = 1024        # hidden size
KC = H // 128   # feature chunks (8)
G3 = 3 * H      # gate width
NCH = G3 // 128  # gate feature chunks (24)
T = 30          # sequences
R = 32          # padded runs per core (30 real + 2 pad)
S = 64          # steps (batch dim acts as sequence length)
NSEG = 8
SUB = 4
NCORES = 8


def build_nc(steps=S, debug=False):
    """Build the per-core Bass module (same program on all 8 cores)."""
    NB = steps // 16          # 512-column blocks in phase A
    OBLK = steps // 8         # 8-step output/gi blocks
    nc = bacc.Bacc()
    if debug:
        dbgi = nc.declare_dram_parameter(
            "dbgi", [128, NCH, NB, 16, R], BF16, isOutput=True)
        dbgh1 = nc.declare_dram_parameter(
            "dbgh1", [steps, 128, KC, R], BF16, isOutput=True)

    xt = nc.declare_dram_parameter("xt", [128, KC, steps * R], BF16, isOutput=False)
    wi0t = nc.declare_dram_parameter("wi0t", [128, KC, G3], BF16, isOutput=False)
    wh0t = nc.declare_dram_parameter("wh0t", [128, KC, G3], BF16, isOutput=False)
    wi1t = nc.declare_dram_parameter("wi1t", [128, KC, G3], BF16, isOutput=False)
    wh1t = nc.declare_dram_parameter("wh1t", [128, KC, G3], BF16, isOutput=False)
    biasA = nc.declare_dram_parameter("biasA", [128, NCH], F32, isOutput=False)
    b2rz = nc.declare_dram_parameter("b2rz", [1, 2 * H], BF16, isOutput=False)
    b2n = nc.declare_dram_parameter("b2n", [1, H], BF16, isOutput=False)
    bhn1 = nc.declare_dram_parameter("bhn1", [1, H], BF16, isOutput=False)
    bhn2 = nc.declare_dram_parameter("bhn2", [1, H], BF16, isOutput=False)
    h1t0 = nc.declare_dram_parameter("h1t0", [128, KC, R], BF16, isOutput=False)
    h2t0 = nc.declare_dram_parameter("h2t0", [128, KC, R], BF16, isOutput=False)
    outp = nc.declare_dram_parameter("out", [OBLK, 128, KC, 8, R], BF16, isOutput=True)

    # gi1, per-step sliceable: [p, mc, nb, sj16, r]
    gi1b = nc.dram_tensor("gi1b", [128, NCH, NB, 16, R], BF16)

    with tile.TileContext(nc) as tc:
        with (
            tc.tile_pool(name="wpool", bufs=1) as wp,
            tc.tile_pool(name="cpool", bufs=1) as cp,
        ):
            # Recurrence weights: issue early on the DVE queue so the
            # transfers fill DMA idle time during phase A.
            wh0s = wp.tile([128, KC, G3], BF16, name="wh0s")
            nc.scalar.dma_start(out=wh0s, in_=wh0t[:])
            wi1s = wp.tile([128, KC, G3], BF16, name="wi1s")
            nc.scalar.dma_start(out=wi1s, in_=wi1t[:])
            wh1s = wp.tile([128, KC, G3], BF16, name="wh1s")
            nc.scalar.dma_start(out=wh1s, in_=wh1t[:])

            identf = cp.tile([128, 128], F32, name="identf")
            make_identity(nc, identf)
            ident = cp.tile([128, 128], BF16, name="ident")
            nc.vector.tensor_copy(ident, identf)
            ones = cp.tile([1, R], BF16, name="ones")
            nc.vector.memset(ones, 1.0)
            bhn1s = cp.tile([1, H], BF16, name="bhn1s")
            nc.scalar.dma_start(out=bhn1s, in_=bhn1[:])
            bhn2s = cp.tile([1, H], BF16, name="bhn2s")
            nc.scalar.dma_start(out=bhn2s, in_=bhn2[:])
            b2rzs = cp.tile([1, 2 * H], BF16, name="b2rzs")
            nc.scalar.dma_start(out=b2rzs, in_=b2rz[:])
            b2ns = cp.tile([1, H], BF16, name="b2ns")
            nc.scalar.dma_start(out=b2ns, in_=b2n[:])
            h1t0s = cp.tile([128, KC, R], BF16, name="h1t0s")
            nc.scalar.dma_start(out=h1t0s, in_=h1t0[:])
            h2t0s = cp.tile([128, KC, R], BF16, name="h2t0s")
            nc.scalar.dma_start(out=h2t0s, in_=h2t0[:])

            # ---------------- Phase A: gi1 = wi0 @ x.T + biases ----------------
            with (
                tc.tile_pool(name="axp", bufs=1) as axp,
                tc.tile_pool(name="awp", bufs=2) as awp,
                tc.tile_pool(name="apsp", bufs=2, space="PSUM") as apsp,
                tc.tile_pool(name="aevp", bufs=3) as aevp,
            ):
                xts = axp.tile([128, KC, steps * R], BF16, name="xts")
                nc.sync.dma_start(out=xts, in_=xt[:])
                biasAs = axp.tile([128, NCH], F32, name="biasAs")
                nc.sync.dma_start(out=biasAs, in_=biasA[:])
                for mc in range(NCH):
                    wc = awp.tile([128, KC, 128], BF16, tag="awc", name=f"awc{mc}")
                    nc.sync.dma_start(out=wc, in_=wi0t[:, :, mc * 128:(mc + 1) * 128])
                    for nb in range(NB):
                        ps = apsp.tile([128, 512], F32, tag="aps", name=f"aps_{mc}_{nb}")
                        for k in range(KC):
                            nc.tensor.matmul(
                                ps, wc[:, k, :], xts[:, k, nb * 512:(nb + 1) * 512],
                                start=(k == 0), stop=(k == KC - 1))
                        ev = aevp.tile([128, 16, R], BF16, tag="aev", name=f"aev_{mc}_{nb}")
                        nc.scalar.activation(
                            ev, ps.rearrange("p (s r) -> p s r", s=16),
                            AF.Identity, bias=biasAs[:, mc:mc + 1])
                        nc.sync.dma_start(out=gi1b[:, mc, nb, :, :], in_=ev)

            # ---------------- Fused step loop (B + C + D) ----------------
            with (
                tc.tile_pool(name="gip", bufs=2) as gip,
                tc.tile_pool(name="obp", bufs=2) as obp,
                tc.tile_pool(name="h1p", bufs=3) as h1p,
                tc.tile_pool(name="gtp", bufs=4) as gtp,
                tc.tile_pool(name="prz1", bufs=1, space="PSUM") as prz1,
                tc.tile_pool(name="pn1", bufs=1, space="PSUM") as pn1,
                tc.tile_pool(name="prz2", bufs=2, space="PSUM") as prz2,
                tc.tile_pool(name="pn2", bufs=2, space="PSUM") as pn2,
                tc.tile_pool(name="pgn2", bufs=1, space="PSUM") as pgn2,
            ):
                def prefetch(bb):
                    g = gip.tile([128, NCH, 8, R], BF16, tag="gi", name=f"gi_{bb}")
                    nb, half = divmod(bb, 2)
                    nc.sync.dma_start(
                        out=g, in_=gi1b[:, :, nb, half * 8:half * 8 + 8, :])
                    return g

                def gates(s, hh, rzp, npp, gin_ap, hold, hout_ap, ln):
                    """One half (4 feature chunks) of GRU gate math."""
                    c0 = 4 * hh
                    rzs = gtp.tile([128, 2, 4, R], BF16, tag=f"rzs{ln}",
                                   name=f"rzs{ln}_{s}_{hh}")
                    nc.scalar.activation(rzs, rzp[:, :, c0:c0 + 4, :], AF.Sigmoid)
                    t1 = gtp.tile([128, 4, R], BF16, tag=f"t1{ln}",
                                  name=f"t1{ln}_{s}_{hh}")
                    nc.vector.tensor_mul(t1, rzs[:, 0], npp[:, c0:c0 + 4, :])
                    npre = gtp.tile([128, 4, R], BF16, tag=f"npre{ln}",
                                    name=f"npre{ln}_{s}_{hh}")
                    nc.vector.tensor_add(npre, t1, gin_ap)
                    nn = gtp.tile([128, 4, R], BF16, tag=f"nn{ln}",
                                  name=f"nn{ln}_{s}_{hh}")
                    nc.scalar.activation(nn, npre, AF.Tanh)
                    dd = gtp.tile([128, 4, R], BF16, tag=f"dd{ln}",
                                  name=f"dd{ln}_{s}_{hh}")
                    nc.vector.tensor_sub(dd, hold[:, c0:c0 + 4, :], nn)
                    t2 = gtp.tile([128, 4, R], BF16, tag=f"t2{ln}",
                                  name=f"t2{ln}_{s}_{hh}")
                    nc.vector.tensor_mul(t2, rzs[:, 1], dd)
                    nc.vector.tensor_add(hout_ap, nn, t2)

                gtile = prefetch(0)
                gnext = None
                h1prev = h1t0s
                h2prev = h2t0s
                ob = None
                for s in range(steps):
                    bb, sj = divmod(s, 8)
                    if sj == 0:
                        if bb > 0:
                            gtile = gnext
                        ob = obp.tile([128, KC, 8, R], BF16, tag="ob",
                                      name=f"ob_{bb}")

                    # ---- layer 1: gh1 (+gi1 rz, +bhn1), gates by halves ----
                    # PSUM zero-region rule: exactly ONE start (first matmul
                    # into the tile) and ONE stop (last matmul into the tile)
                    # per step -- a second start=True would mark the whole 2KB
                    # zero region pending-zero and destroy sibling partials.
                    rz1 = prz1.tile([128, 2, KC, R], F32, tag="rz1", name=f"rz1_{s}")
                    n1 = pn1.tile([128, KC, R], F32, tag="n1", name=f"n1_{s}")
                    h1new = h1p.tile([128, KC, R], BF16, tag="h1", name=f"h1_{s}")
                    for hh in range(2):
                        c0 = 4 * hh
                        for g in range(2):
                            for c in range(c0, c0 + 4):
                                n = g * 8 + c
                                for k in range(KC):
                                    nc.tensor.matmul(
                                        rz1[:, g, c, :],
                                        wh0s[:, k, n * 128:(n + 1) * 128],
                                        h1prev[:, k, :],
                                        start=(hh == 0 and g == 0 and c == 0
                                               and k == 0), stop=False)
                                nc.tensor.matmul(
                                    rz1[:, g, c, :], ident, gtile[:, n, sj, :],
                                    start=False,
                                    stop=(hh == 1 and g == 1 and c == 7))
                        for c in range(c0, c0 + 4):
                            n = 16 + c
                            for k in range(KC):
                                nc.tensor.matmul(
                                    n1[:, c, :], wh0s[:, k, n * 128:(n + 1) * 128],
                                    h1prev[:, k, :],
                                    start=(hh == 0 and c == 0 and k == 0),
                                    stop=False)
                            nc.tensor.matmul(
                                n1[:, c, :], bhn1s[0:1, c * 128:(c + 1) * 128],
                                ones, start=False, stop=(hh == 1 and c == 7))
                        gates(s, hh, rz1, n1, gtile[:, 16 + c0:16 + c0 + 4, sj, :],
                              h1prev, h1new[:, c0:c0 + 4, :], "a")

                    # ---- layer 2 recurrent gh2 (k-half split; h2prev) ----
                    rz2 = prz2.tile([128, 2, KC, R], F32, tag="rz2", name=f"rz2_{s}")
                    n2 = pn2.tile([128, KC, R], F32, tag="n2", name=f"n2_{s}")
                    gn2 = pgn2.tile([128, KC, R], F32, tag="gn2", name=f"gn2_{s}")
                    for kh in range(2):
                        for g in range(2):
                            for c in range(KC):
                                n = g * 8 + c
                                for k in range(4 * kh, 4 * kh + 4):
                                    nc.tensor.matmul(
                                        rz2[:, g, c, :],
                                        wh1s[:, k, n * 128:(n + 1) * 128],
                                        h2prev[:, k, :],
                                        start=(kh == 0 and g == 0 and c == 0
                                               and k == 0), stop=False)
                        for c in range(KC):
                            n = 16 + c
                            for k in range(4 * kh, 4 * kh + 4):
                                nc.tensor.matmul(
                                    n2[:, c, :], wh1s[:, k, n * 128:(n + 1) * 128],
                                    h2prev[:, k, :],
                                    start=(kh == 0 and c == 0 and k == 0),
                                    stop=False)
                    for g in range(2):
                        for c in range(KC):
                            n = g * 8 + c
                            nc.tensor.matmul(
                                rz2[:, g, c, :], b2rzs[0:1, n * 128:(n + 1) * 128],
                                ones, start=False, stop=False)
                    for c in range(KC):
                        nc.tensor.matmul(
                            n2[:, c, :], bhn2s[0:1, c * 128:(c + 1) * 128],
                            ones, start=False, stop=(c == 7))
                    for c in range(KC):
                        nc.tensor.matmul(
                            gn2[:, c, :], b2ns[0:1, c * 128:(c + 1) * 128],
                            ones, start=(c == 0), stop=False)

                    # ---- gi2 = wi1 @ h1new, accumulated into rz2 / gn2 ----
                    for kh in range(2):
                        for g in range(2):
                            for c in range(KC):
                                n = g * 8 + c
                                for k in range(4 * kh, 4 * kh + 4):
                                    nc.tensor.matmul(
                                        rz2[:, g, c, :],
                                        wi1s[:, k, n * 128:(n + 1) * 128],
                                        h1new[:, k, :],
                                        start=False,
                                        stop=(kh == 1 and g == 1 and c == 7
                                              and k == 7))
                        for c in range(KC):
                            n = 16 + c
                            for k in range(4 * kh, 4 * kh + 4):
                                nc.tensor.matmul(
                                    gn2[:, c, :], wi1s[:, k, n * 128:(n + 1) * 128],
                                    h1new[:, k, :],
                                    start=False,
                                    stop=(kh == 1 and c == 7 and k == 7))

                    if sj == 0 and bb + 1 < OBLK:
                        gnext = prefetch(bb + 1)

                    # ---- layer 2 gates -> output staging ----
                    for hh in range(2):
                        gates(s, hh, rz2, n2, gn2[:, 4 * hh:4 * hh + 4, :],
                              h2prev, ob[:, 4 * hh:4 * hh + 4, sj, :], "b")

                    h2prev = ob[:, :, sj, :]
                    if debug:
                        nc.sync.dma_start(out=dbgh1[s], in_=h1new)
                    h1prev = h1new
                    if sj == 7:
                        nc.sync.dma_start(out=outp[bb], in_=ob)
                if debug:
                    dcp = gip.tile([128, NCH, 8, R], BF16, tag="gi", name="dcp")
                    for nb in range(NB):
                        for hf in range(2):
                            nc.sync.dma_start(
                                out=dcp, in_=gi1b[:, :, nb, hf * 8:hf * 8 + 8, :])
                            nc.sync.dma_start(
                                out=dbgi[:, :, nb, hf * 8:hf * 8 + 8, :], in_=dcp)

    nc.finalize()
    return nc


def ode_traj(w1, b1, w2, b2, w3, b3):
    """RK4 trajectory of the ODE, mirroring the reference exactly (fp32)."""
    w1t = w1.T.astype(np.float32)
    w2t = w2.T.astype(np.float32)
    w3t = w3.T.astype(np.float32)

    def f(h):
        a = np.tanh(h @ w1t + b1)
        a = np.tanh(a @ w2t + b2)
        return a @ w3t + b3

    dt = np.float32((1.0 / NSEG) / SUB)
    h = np.zeros((2, H), np.float32)
    traj = []
    for _ in range(NSEG):
        for _ in range(SUB):
            k1 = f(h)
            k2 = f(h + np.float32(0.5) * dt * k1)
            k3 = f(h + np.float32(0.5) * dt * k2)
            k4 = f(h + dt * k3)
            h = h + (dt / np.float32(6.0)) * (k1 + np.float32(2.0) * k2
                                              + np.float32(2.0) * k3 + k4)
        traj.append(h.copy())
    return np.stack(traj)  # (NSEG, 2, H)


def _wt(w):
    """[G3, H] weight -> feature-major [128, KC, G3] bf16."""
    return np.ascontiguousarray(
        w.T.reshape(KC, 128, G3).transpose(1, 0, 2)).astype(NP_BF16)


def make_in_maps(x, w1, b1, w2, b2, w3, b3, wi0, wh0, bi0, bh0,
                 wi1, wh1, bi1, bh1, steps=S, cores=NCORES):
    traj = ode_traj(w1, b1, w2, b2, w3, b3)

    # xt[p, k, s*R + r] = x[s, r, k*128+p]
    xp = np.zeros((128, KC, steps, R), np.float32)
    xp[:, :, :, :T] = x[:steps].reshape(steps, T, KC, 128).transpose(3, 2, 0, 1)
    xt = xp.reshape(128, KC, steps * R).astype(NP_BF16)

    biasA = np.concatenate([bi0[:2 * H] + bh0[:2 * H], bi0[2 * H:]])
    shared = {
        "xt": xt,
        "wi0t": _wt(wi0), "wh0t": _wt(wh0),
        "wi1t": _wt(wi1), "wh1t": _wt(wh1),
        "biasA": np.ascontiguousarray(
            biasA.reshape(NCH, 128).T).astype(np.float32),
        "b2rz": (bi1[:2 * H] + bh1[:2 * H]).reshape(1, 2 * H).astype(NP_BF16),
        "b2n": bi1[2 * H:].reshape(1, H).astype(NP_BF16),
        "bhn1": bh0[2 * H:].reshape(1, H).astype(NP_BF16),
        "bhn2": bh1[2 * H:].reshape(1, H).astype(NP_BF16),
    }
    in_maps = []
    for i in range(cores):
        m = dict(shared)
        for nm, vec in (("h1t0", traj[i, 0]), ("h2t0", traj[i, 1])):
            ht = np.broadcast_to(
                vec.reshape(KC, 128).T[:, :, None], (128, KC, R))
            m[nm] = np.ascontiguousarray(ht).astype(NP_BF16)
        in_maps.append(m)
    return in_maps


_NC_CACHE = {}


def _get_nc(steps):
    if steps not in _NC_CACHE:
        _NC_CACHE[steps] = build_nc(steps)
    return _NC_CACHE[steps]


def run_cores(inputs, steps=S, cores=NCORES, **run_kwargs):
    in_maps = make_in_maps(steps=steps, cores=cores, **inputs)
    nc = _get_nc(steps)
    return run_bass_kernel_spmd(nc, in_maps, core_ids=list(range(cores)),
                                **run_kwargs)


def kernel(x, w1, b1, w2, b2, w3, b3, wi0, wh0, bi0, bh0,
           wi1, wh1, bi1, bh1):
    args = dict(x=x, w1=w1, b1=b1, w2=w2, b2=b2, w3=w3, b3=b3,
                wi0=wi0, wh0=wh0, bi0=bi0, bh0=bh0,
                wi1=wi1, wh1=wh1, bi1=bi1, bh1=bh1)
    args = {k: np.asarray(v, np.float32) for k, v in args.items()}
    res = run_cores(args, steps=S, cores=NCORES)
    full = np.empty((S, T * NCORES, H), np.float32)
    for i in range(NCORES):
        o = np.asarray(res.results[i]["out"]).astype(np.float32)
        # [ob, p, k, sj, r] -> [s, r, feat]
        o = o.transpose(0, 3, 4, 2, 1).reshape(S, R, H)
        full[:, i::NCORES, :] = o[:, :T, :]
    return full


# revision 35
# speedup vs baseline: 3.0496x; 1.0314x over previous
"""ODE-RNN Trainium2 kernel (v2: feature-major bf16 formulation).

Problem: out[b, t*8+i, :] = 2-layer GRU (H=1024) run over the batch dim
(64 steps) of sequence t (30 sequences), with initial hiddens taken from an
RK4-integrated ODE trajectory (8 grid points, shared across all runs).

Strategy (8 NeuronCores, pure data-parallel, no collectives):
  - ODE trajectory on host (tiny, exactly mirrors reference math).
  - Core i handles the 30 GRU runs with init traj[i]; R=32 padded runs.
  - Everything on-device is kept FEATURE-MAJOR (transposed): state tensors
    live as [128 partitions = feature-within-chunk, KC=8 chunks, R runs].
    The recurrent matmuls then use the (resident, bf16) weights as the
    128x128 PE-stationary operand and stream the small [128, R] state as
    the moving operand: cost-model rows per step drop 4x vs streaming the
    weights, and no PE transposes are needed anywhere.
  - Phase A: gi1 = wi0 @ x.T + bias as a dense GEMM, output to DRAM in a
    per-step-sliceable layout (bf16).
  - Fused step loop (phases B+C+D): per step s, layer-1 gates from
    (gi1[s], h1[s-1]); gi2 = wi1 @ h1[s] accumulated directly into the
    layer-2 gate PSUM; layer-2 gates -> h2[s] -> output staging buffer.
    All biases are accumulated on the PE (identity / ones-row matmuls),
    so the vector/scalar engines only do the nonlinear gate math.
  - Emission is ordered so the PE never waits on the gate-math chains:
    layer-1 gate math overlaps layer-2 recurrent matmuls, and the fresh
    states are consumed k-half by k-half.
"""

import numpy as np

try:
    import concourse.bass as bass  # noqa: F401
except ImportError:  # pragma: no cover - fallback for bare environments
    import sys
    sys.path.insert(0, "/opt/trn_rl_repo")
    import concourse.bass as bass  # noqa: F401

import concourse.mybir as mybir
import concourse.tile as tile
from concourse import bacc
from concourse.bass_utils import run_bass_kernel_spmd
from concourse.masks import make_identity

F32 = mybir.dt.float32
BF16 = mybir.dt.float16  # fp16: same PE cost as bf16, more mantissa
NP_BF16 = mybir.dt.np(mybir.dt.float16)
AF = mybir.ActivationFunctionType

H = 1024        # hidden size
KC = H // 128   # feature chunks (8)
G3 = 3 * H      # gate width
NCH = G3 // 128  # gate feature chunks (24)
T = 30          # sequences
R = 32          # padded runs per core (30 real + 2 pad)
S = 64          # steps (batch dim acts as sequence length)
NSEG = 8
SUB = 4
NCORES = 8


def build_nc(steps=S, debug=False):
    """Build the per-core Bass module (same program on all 8 cores)."""
    NB = steps // 16          # 512-column blocks in phase A
    OBLK = steps // 8         # 8-step output/gi blocks
    nc = bacc.Bacc()
    if debug:
        dbgi = nc.declare_dram_parameter(
            "dbgi", [128, NCH, NB, 16, R], BF16, isOutput=True)
        dbgh1 = nc.declare_dram_parameter(
            "dbgh1", [steps, 128, KC, R], BF16, isOutput=True)

    xt = nc.declare_dram_parameter("xt", [128, steps // 8, KC, 256], BF16,
                               isOutput=False)
    wi0t = nc.declare_dram_parameter("wi0t", [128, 4, KC, G3 // 4], BF16,
                                 isOutput=False)
    wh0t = nc.declare_dram_parameter("wh0t", [128, KC, G3], BF16, isOutput=False)
    wi1t = nc.declare_dram_parameter("wi1t", [128, KC, G3], BF16, isOutput=False)
    wh1t = nc.declare_dram_parameter("wh1t", [128, KC, G3], BF16, isOutput=False)
    biasA = nc.declare_dram_parameter("biasA", [128, NCH], F32, isOutput=False)
    b2rz = nc.declare_dram_parameter("b2rz", [1, 2 * H], BF16, isOutput=False)
    b2n = nc.declare_dram_parameter("b2n", [1, H], BF16, isOutput=False)
    bhn1 = nc.declare_dram_parameter("bhn1", [1, H], BF16, isOutput=False)
    bhn2 = nc.declare_dram_parameter("bhn2", [1, H], BF16, isOutput=False)
    h1t0 = nc.declare_dram_parameter("h1t0", [128, KC, R], BF16, isOutput=False)
    h2t0 = nc.declare_dram_parameter("h2t0", [128, KC, R], BF16, isOutput=False)
    outp = nc.declare_dram_parameter("out", [OBLK, 128, KC, 8, R], BF16, isOutput=True)

    # gi1, per-step sliceable: [p, mc, sj8, r]; one tensor per 8-step
    # block so each step-loop prefetch only depends on its own A block.
    gi1b = [nc.dram_tensor(f"gi1b{bb}", [128, NCH, 8, R], BF16)
            for bb in range(OBLK)]

    with tile.TileContext(nc) as tc:
        with (
            tc.tile_pool(name="wpool", bufs=1) as wp,
        ):
            # Recurrence weights: tiles allocated here; their DMAs are
            # interleaved with phase A's input loads below so the x/wi0
            # chunks phase A needs first reach the DMA engines first.
            wh0s = wp.tile([128, KC, G3], BF16, name="wh0s")
            wi1s = wp.tile([128, KC, G3], BF16, name="wi1s")
            wh1s = wp.tile([128, KC, G3], BF16, name="wh1s")

            # ---------------- Phase A: gi1 = wi0 @ x.T + biases ----------------
            with (
                tc.tile_pool(name="axp", bufs=1) as axp,
                tc.tile_pool(name="axcp", bufs=2) as axcp,
                tc.tile_pool(name="apsp", bufs=6, space="PSUM") as apsp,
                tc.tile_pool(name="aevp", bufs=12) as aevp,
            ):
                # wi0 in quarters (separate tiles -> finer DMA deps, so the
                # first matmuls only wait on quarter 0).
                wi0q = [axp.tile([128, KC, G3 // 4], BF16, name=f"wi0q{q}")
                        for q in range(4)]
                biasAs = axp.tile([128, NCH], F32, name="biasAs")
                # SP queue carries only the no-wait input loads, in the order
                # A consumes them; gi1b writes ride the ACT queue behind the
                # ev activations that produce them; the big recurrence-weight
                # loads join the ACT queue mid-A to fill DMA idle time.
                xc0 = axcp.tile([128, KC, 256], BF16, tag="xc", name="xc0")
                nc.sync.dma_start(out=xc0, in_=xt[:, 0])
                nc.sync.dma_start(out=wi0q[0], in_=wi0t[:, 0])
                nc.sync.dma_start(out=biasAs, in_=biasA[:])
                for q in range(1, 4):
                    nc.sync.dma_start(out=wi0q[q], in_=wi0t[:, q])
                for bb in range(OBLK):
                    if bb == 0:
                        xc = xc0
                    else:
                        xc = axcp.tile([128, KC, 256], BF16, tag="xc",
                                       name=f"xc{bb}")
                        nc.sync.dma_start(out=xc, in_=xt[:, bb])
                    if bb == 2:
                        # Quarter-sized weight loads, each pinned (via
                        # wait_until) into a staggered slot so they fill DMA
                        # idle time without displacing A's input loads or
                        # blocking the gi1b write stream for 17us at a time.
                        for wqi, wdst, wsrc in (
                                [(i, wh0s, wh0t) for i in range(4)]
                                + [(4 + i, wi1s, wi1t) for i in range(4)]
                                + [(8 + i, wh1s, wh1t) for i in range(4)]):
                            q = wqi % 4
                            cols = slice(q * (G3 // 4), (q + 1) * (G3 // 4))
                            with tc.tile_wait_until(0.022 + 0.0065 * wqi):
                                nc.scalar.dma_start(
                                    out=wdst[:, :, cols], in_=wsrc[:, :, cols])
                    for mc in range(NCH):
                        ps = apsp.tile([128, 256], F32, tag="aps", name=f"aps_{mc}_{bb}")
                        for k in range(KC):
                            q, qo = divmod(mc, 6)
                            nc.tensor.matmul(
                                ps, wi0q[q][:, k, qo * 128:(qo + 1) * 128],
                                xc[:, k, :],
                                start=(k == 0), stop=(k == KC - 1))
                        ev = aevp.tile([128, 8, R], BF16, tag="aev", name=f"aev_{mc}_{bb}")
                        nc.vector.tensor_scalar_add(
                            ev, ps.rearrange("p (s r) -> p s r", s=8),
                            biasAs[:, mc:mc + 1])
                        nc.scalar.dma_start(out=gi1b[bb][:, mc, :, :], in_=ev)

            # ---------------- Fused step loop (B + C + D) ----------------
            with (
                tc.tile_pool(name="cpool", bufs=1) as cp,
                tc.tile_pool(name="gip", bufs=2) as gip,
                tc.tile_pool(name="obp", bufs=3) as obp,
                tc.tile_pool(name="h1p", bufs=4) as h1p,
                tc.tile_pool(name="gtp", bufs=4) as gtp,
                tc.tile_pool(name="prz1", bufs=2, space="PSUM") as prz1,
                tc.tile_pool(name="pn1", bufs=1, space="PSUM") as pn1,
                tc.tile_pool(name="prz2", bufs=2, space="PSUM") as prz2,
                tc.tile_pool(name="pn2", bufs=1, space="PSUM") as pn2,
                tc.tile_pool(name="pgn2", bufs=2, space="PSUM") as pgn2,
            ):
                identf = cp.tile([128, 128], F32, name="identf")
                make_identity(nc, identf)
                ident = cp.tile([128, 128], BF16, name="ident")
                nc.vector.tensor_copy(ident, identf)
                ones = cp.tile([1, R], BF16, name="ones")
                nc.vector.memset(ones, 1.0)
                bhn1s = cp.tile([1, H], BF16, name="bhn1s")
                nc.sync.dma_start(out=bhn1s, in_=bhn1[:])
                bhn2s = cp.tile([1, H], BF16, name="bhn2s")
                nc.sync.dma_start(out=bhn2s, in_=bhn2[:])
                b2rzs = cp.tile([1, 2 * H], BF16, name="b2rzs")
                nc.sync.dma_start(out=b2rzs, in_=b2rz[:])
                b2ns = cp.tile([1, H], BF16, name="b2ns")
                nc.sync.dma_start(out=b2ns, in_=b2n[:])
                h1t0s = cp.tile([128, KC, R], BF16, name="h1t0s")
                nc.sync.dma_start(out=h1t0s, in_=h1t0[:])
                h2t0s = cp.tile([128, KC, R], BF16, name="h2t0s")
                nc.sync.dma_start(out=h2t0s, in_=h2t0[:])

                def prefetch(bb):
                    # gpsimd (SWDGE) queue: independent of the SP queue that
                    # carries phase A's gi1b writes, so each prefetch fires as
                    # soon as its own 8-step block's writes complete.
                    g = gip.tile([128, NCH, 8, R], BF16, tag="gi", name=f"gi_{bb}")
                    nc.gpsimd.dma_start(out=g, in_=gi1b[bb][:])
                    return g

                def gates(s, hh, rzp, npp, gin_ap, hold, hout_ap, ln):
                    """One half (4 feature chunks) of GRU gate math."""
                    c0 = 4 * hh
                    rzs = gtp.tile([128, 2, 4, R], BF16, tag=f"rzs{ln}",
                                   name=f"rzs{ln}_{s}_{hh}")
                    nc.scalar.activation(rzs, rzp[:, :, c0:c0 + 4, :], AF.Sigmoid)
                    t1 = gtp.tile([128, 4, R], BF16, tag=f"t1{ln}",
                                  name=f"t1{ln}_{s}_{hh}")
                    nc.vector.tensor_mul(t1, rzs[:, 0], npp[:, c0:c0 + 4, :])
                    npre = gtp.tile([128, 4, R], BF16, tag=f"npre{ln}",
                                    name=f"npre{ln}_{s}_{hh}")
                    nc.vector.tensor_add(npre, t1, gin_ap)
                    nn = gtp.tile([128, 4, R], BF16, tag=f"nn{ln}",
                                  name=f"nn{ln}_{s}_{hh}")
                    nc.scalar.activation(nn, npre, AF.Tanh)
                    dd = gtp.tile([128, 4, R], BF16, tag=f"dd{ln}",
                                  name=f"dd{ln}_{s}_{hh}")
                    nc.vector.tensor_sub(dd, hold[:, c0:c0 + 4, :], nn)
                    t2 = gtp.tile([128, 4, R], BF16, tag=f"t2{ln}",
                                  name=f"t2{ln}_{s}_{hh}")
                    nc.vector.tensor_mul(t2, rzs[:, 1], dd)
                    nc.vector.tensor_add(hout_ap, nn, t2)

                gtile = prefetch(0)
                gnext = None
                h1prev = h1t0s
                h2prev = h2t0s
                ob = None
                for s in range(steps):
                    bb, sj = divmod(s, 8)
                    if sj == 0:
                        if bb > 0:
                            gtile = gnext
                        ob = obp.tile([128, KC, 8, R], BF16, tag="ob",
                                      name=f"ob_{bb}")

                    # ---- layer 1: gh1 (+gi1 rz, +bhn1), gates by halves ----
                    # PSUM zero-region rule: exactly ONE start (first matmul
                    # into the tile) and ONE stop (last matmul into the tile)
                    # per step -- a second start=True would mark the whole 2KB
                    # zero region pending-zero and destroy sibling partials.
                    rz1 = prz1.tile([128, 2, KC, R], F32, tag="rz1", name=f"rz1_{s}")
                    n1 = pn1.tile([128, KC, R], F32, tag="n1", name=f"n1_{s}")
                    h1new = h1p.tile([128, KC, R], BF16, tag="h1", name=f"h1_{s}")
                    for hh in range(2):
                        c0 = 4 * hh
                        for g in range(2):
                            for c in range(c0, c0 + 4):
                                n = g * 8 + c
                                for k in range(KC):
                                    nc.tensor.matmul(
                                        rz1[:, g, c, :],
                                        wh0s[:, k, n * 128:(n + 1) * 128],
                                        h1prev[:, k, :],
                                        start=(hh == 0 and g == 0 and c == 0
                                               and k == 0), stop=False)
                                nc.tensor.matmul(
                                    rz1[:, g, c, :], ident, gtile[:, n, sj, :],
                                    start=False,
                                    stop=(hh == 1 and g == 1 and c == 7))
                        for c in range(c0, c0 + 4):
                            n = 16 + c
                            for k in range(KC):
                                nc.tensor.matmul(
                                    n1[:, c, :], wh0s[:, k, n * 128:(n + 1) * 128],
                                    h1prev[:, k, :],
                                    start=(hh == 0 and c == 0 and k == 0),
                                    stop=False)
                            nc.tensor.matmul(
                                n1[:, c, :], bhn1s[0:1, c * 128:(c + 1) * 128],
                                ones, start=False, stop=(hh == 1 and c == 7))
                        gates(s, hh, rz1, n1, gtile[:, 16 + c0:16 + c0 + 4, sj, :],
                              h1prev, h1new[:, c0:c0 + 4, :], "a")

                    # ---- layer 2 recurrent gh2 (k-half split; h2prev) ----
                    rz2 = prz2.tile([128, 2, KC, R], F32, tag="rz2", name=f"rz2_{s}")
                    n2 = pn2.tile([128, KC, R], F32, tag="n2", name=f"n2_{s}")
                    gn2 = pgn2.tile([128, KC, R], F32, tag="gn2", name=f"gn2_{s}")
                    for kh in range(2):
                        for g in range(2):
                            for c in range(KC):
                                n = g * 8 + c
                                for k in range(4 * kh, 4 * kh + 4):
                                    nc.tensor.matmul(
                                        rz2[:, g, c, :],
                                        wh1s[:, k, n * 128:(n + 1) * 128],
                                        h2prev[:, k, :],
                                        start=(kh == 0 and g == 0 and c == 0
                                               and k == 0), stop=False)
                        for c in range(KC):
                            n = 16 + c
                            for k in range(4 * kh, 4 * kh + 4):
                                nc.tensor.matmul(
                                    n2[:, c, :], wh1s[:, k, n * 128:(n + 1) * 128],
                                    h2prev[:, k, :],
                                    start=(kh == 0 and c == 0 and k == 0),
                                    stop=False)
                    for g in range(2):
                        for c in range(KC):
                            n = g * 8 + c
                            nc.tensor.matmul(
                                rz2[:, g, c, :], b2rzs[0:1, n * 128:(n + 1) * 128],
                                ones, start=False, stop=False)
                    for c in range(KC):
                        nc.tensor.matmul(
                            n2[:, c, :], bhn2s[0:1, c * 128:(c + 1) * 128],
                            ones, start=False, stop=(c == 7))

                    # ---- gi2 = wi1 @ h1new, accumulated into rz2 / gn2 ----
                    # gn2's group opener is the first gi2 matmul (which waits
                    # on h1new anyway); a dep-free opener like the b2n bias
                    # matmul gets hoisted by the scheduler and then HOL-blocks
                    # the PE queue on its PSUM WAR wait.
                    for kh in range(2):
                        for g in range(2):
                            for c in range(KC):
                                n = g * 8 + c
                                for k in range(4 * kh, 4 * kh + 4):
                                    nc.tensor.matmul(
                                        rz2[:, g, c, :],
                                        wi1s[:, k, n * 128:(n + 1) * 128],
                                        h1new[:, k, :],
                                        start=False,
                                        stop=(kh == 1 and g == 1 and c == 7
                                              and k == 7))
                        for c in range(KC):
                            n = 16 + c
                            for k in range(4 * kh, 4 * kh + 4):
                                nc.tensor.matmul(
                                    gn2[:, c, :], wi1s[:, k, n * 128:(n + 1) * 128],
                                    h1new[:, k, :],
                                    start=(kh == 0 and c == 0 and k == 0),
                                    stop=False)
                    for c in range(KC):
                        nc.tensor.matmul(
                            gn2[:, c, :], b2ns[0:1, c * 128:(c + 1) * 128],
                            ones, start=False, stop=(c == 7))

                    if sj == 0 and bb + 1 < OBLK:
                        gnext = prefetch(bb + 1)

                    # ---- layer 2 gates -> output staging ----
                    for hh in range(2):
                        gates(s, hh, rz2, n2, gn2[:, 4 * hh:4 * hh + 4, :],
                              h2prev, ob[:, 4 * hh:4 * hh + 4, sj, :], "b")

                    h2prev = ob[:, :, sj, :]
                    if debug:
                        nc.sync.dma_start(out=dbgh1[s], in_=h1new)
                    h1prev = h1new
                    if sj == 7:
                        nc.sync.dma_start(out=outp[bb], in_=ob)
                if debug:
                    dcp = gip.tile([128, NCH, 8, R], BF16, tag="gi", name="dcp")
                    for nb in range(NB):
                        for hf in range(2):
                            nc.sync.dma_start(
                                out=dcp, in_=gi1b[nb][:, :, hf * 8:hf * 8 + 8, :])
                            nc.sync.dma_start(
                                out=dbgi[:, :, nb, hf * 8:hf * 8 + 8, :], in_=dcp)

    nc.finalize()
    return nc


def ode_traj(w1, b1, w2, b2, w3, b3):
    """RK4 trajectory of the ODE, mirroring the reference exactly (fp32)."""
    w1t = w1.T.astype(np.float32)
    w2t = w2.T.astype(np.float32)
    w3t = w3.T.astype(np.float32)

    def f(h):
        a = np.tanh(h @ w1t + b1)
        a = np.tanh(a @ w2t + b2)
        return a @ w3t + b3

    dt = np.float32((1.0 / NSEG) / SUB)
    h = np.zeros((2, H), np.float32)
    traj = []
    for _ in range(NSEG):
        for _ in range(SUB):
            k1 = f(h)
            k2 = f(h + np.float32(0.5) * dt * k1)
            k3 = f(h + np.float32(0.5) * dt * k2)
            k4 = f(h + dt * k3)
            h = h + (dt / np.float32(6.0)) * (k1 + np.float32(2.0) * k2
                                              + np.float32(2.0) * k3 + k4)
        traj.append(h.copy())
    return np.stack(traj)  # (NSEG, 2, H)


def _wt(w):
    """[G3, H] weight -> feature-major [128, KC, G3] bf16."""
    return np.ascontiguousarray(
        w.T.reshape(KC, 128, G3).transpose(1, 0, 2)).astype(NP_BF16)


def make_in_maps(x, w1, b1, w2, b2, w3, b3, wi0, wh0, bi0, bh0,
                 wi1, wh1, bi1, bh1, steps=S, cores=NCORES):
    traj = ode_traj(w1, b1, w2, b2, w3, b3)

    # xt[p, bb, k, sj*R + r] = x[bb*8+sj, r, k*128+p]
    xp = np.zeros((128, KC, steps, R), np.float32)
    xp[:, :, :, :T] = x[:steps].reshape(steps, T, KC, 128).transpose(3, 2, 0, 1)
    xt = np.ascontiguousarray(
        xp.reshape(128, KC, steps // 8, 8 * R).transpose(0, 2, 1, 3)
    ).astype(NP_BF16)

    biasA = np.concatenate([bi0[:2 * H] + bh0[:2 * H], bi0[2 * H:]])
    wi0q = np.ascontiguousarray(
        _wt(wi0).reshape(128, KC, 4, G3 // 4).transpose(0, 2, 1, 3))
    shared = {
        "xt": xt,
        "wi0t": wi0q, "wh0t": _wt(wh0),
        "wi1t": _wt(wi1), "wh1t": _wt(wh1),
        "biasA": np.ascontiguousarray(
            biasA.reshape(NCH, 128).T).astype(np.float32),
        "b2rz": (bi1[:2 * H] + bh1[:2 * H]).reshape(1, 2 * H).astype(NP_BF16),
        "b2n": bi1[2 * H:].reshape(1, H).astype(NP_BF16),
        "bhn1": bh0[2 * H:].reshape(1, H).astype(NP_BF16),
        "bhn2": bh1[2 * H:].reshape(1, H).astype(NP_BF16),
    }
    in_maps = []
    for i in range(cores):
        m = dict(shared)
        for nm, vec in (("h1t0", traj[i, 0]), ("h2t0", traj[i, 1])):
            ht = np.broadcast_to(
                vec.reshape(KC, 128).T[:, :, None], (128, KC, R))
            m[nm] = np.ascontiguousarray(ht).astype(NP_BF16)
        in_maps.append(m)
    return in_maps


_NC_CACHE = {}


def _get_nc(steps):
    if steps not in _NC_CACHE:
        _NC_CACHE[steps] = build_nc(steps)
    return _NC_CACHE[steps]


def run_cores(inputs, steps=S, cores=NCORES, **run_kwargs):
    in_maps = make_in_maps(steps=steps, cores=cores, **inputs)
    nc = _get_nc(steps)
    return run_bass_kernel_spmd(nc, in_maps, core_ids=list(range(cores)),
                                **run_kwargs)


def kernel(x, w1, b1, w2, b2, w3, b3, wi0, wh0, bi0, bh0,
           wi1, wh1, bi1, bh1):
    args = dict(x=x, w1=w1, b1=b1, w2=w2, b2=b2, w3=w3, b3=b3,
                wi0=wi0, wh0=wh0, bi0=bi0, bh0=bh0,
                wi1=wi1, wh1=wh1, bi1=bi1, bh1=bh1)
    args = {k: np.asarray(v, np.float32) for k, v in args.items()}
    res = run_cores(args, steps=S, cores=NCORES)
    full = np.empty((S, T * NCORES, H), np.float32)
    for i in range(NCORES):
        o = np.asarray(res.results[i]["out"]).astype(np.float32)
        # [ob, p, k, sj, r] -> [s, r, feat]
        o = o.transpose(0, 3, 4, 2, 1).reshape(S, R, H)
        full[:, i::NCORES, :] = o[:, :T, :]
    return full


# revision 49
# speedup vs baseline: 3.4291x; 1.1245x over previous
"""ODE-RNN Trainium2 kernel (v2: feature-major bf16 formulation).

Problem: out[b, t*8+i, :] = 2-layer GRU (H=1024) run over the batch dim
(64 steps) of sequence t (30 sequences), with initial hiddens taken from an
RK4-integrated ODE trajectory (8 grid points, shared across all runs).

Strategy (8 NeuronCores, pure data-parallel, no collectives):
  - ODE trajectory on host (tiny, exactly mirrors reference math).
  - Core i handles the 30 GRU runs with init traj[i]; R=32 padded runs.
  - Everything on-device is kept FEATURE-MAJOR (transposed): state tensors
    live as [128 partitions = feature-within-chunk, KC=8 chunks, R runs].
    The recurrent matmuls then use the (resident, bf16) weights as the
    128x128 PE-stationary operand and stream the small [128, R] state as
    the moving operand: cost-model rows per step drop 4x vs streaming the
    weights, and no PE transposes are needed anywhere.
  - Phase A: gi1 = wi0 @ x.T + bias as a dense GEMM, output to DRAM in a
    per-step-sliceable layout (bf16).
  - Fused step loop (phases B+C+D): per step s, layer-1 gates from
    (gi1[s], h1[s-1]); gi2 = wi1 @ h1[s] accumulated directly into the
    layer-2 gate PSUM; layer-2 gates -> h2[s] -> output staging buffer.
    All biases are accumulated on the PE (identity / ones-row matmuls),
    so the vector/scalar engines only do the nonlinear gate math.
  - Emission is ordered so the PE never waits on the gate-math chains:
    layer-1 gate math overlaps layer-2 recurrent matmuls, and the fresh
    states are consumed k-half by k-half.
"""

import numpy as np

try:
    import concourse.bass as bass  # noqa: F401
except ImportError:  # pragma: no cover - fallback for bare environments
    import sys
    sys.path.insert(0, "/opt/trn_rl_repo")
    import concourse.bass as bass  # noqa: F401

import concourse.mybir as mybir
import concourse.tile as tile
from concourse import bacc
from concourse.bass_utils import run_bass_kernel_spmd
from concourse.masks import make_identity

F32 = mybir.dt.float32
FP8 = mybir.dt.float8e4
NP_FP8 = mybir.dt.np(mybir.dt.float8e4)
DR = mybir.MatmulPerfMode.DoubleRow
SCL = 16.0  # fp8 weight pre-scale; folded back via ACT scale=1/SCL
BF16 = mybir.dt.float16  # fp16: same PE cost as bf16, more mantissa
NP_BF16 = mybir.dt.np(mybir.dt.float16)
AF = mybir.ActivationFunctionType

H = 1024        # hidden size
KC = H // 128   # feature chunks (8)
G3 = 3 * H      # gate width
NCH = G3 // 128  # gate feature chunks (24)
T = 30          # sequences
R = 32          # padded runs per core (30 real + 2 pad)
S = 64          # steps (batch dim acts as sequence length)
NSEG = 8
SUB = 4
NCORES = 8


def build_nc(steps=S, debug=False):
    """Build the per-core Bass module (same program on all 8 cores)."""
    NB = steps // 16          # 512-column blocks in phase A
    OBLK = steps // 8         # 8-step output/gi blocks
    nc = bacc.Bacc()
    if debug:
        dbgi = nc.declare_dram_parameter(
            "dbgi", [128, NCH, NB, 16, R], BF16, isOutput=True)
        dbgh1 = nc.declare_dram_parameter(
            "dbgh1", [steps, 128, KC, R], BF16, isOutput=True)

    xt = nc.declare_dram_parameter("xt", [128, steps // 8, KC, 256], BF16,
                               isOutput=False)
    wi0t = nc.declare_dram_parameter("wi0t", [128, 4, KC, G3 // 4], BF16,
                                 isOutput=False)
    wh0t = nc.declare_dram_parameter("wh0t", [128, KC, G3], FP8, isOutput=False)
    wi1t = nc.declare_dram_parameter("wi1t", [128, KC, G3], FP8, isOutput=False)
    wh1t = nc.declare_dram_parameter("wh1t", [128, KC, G3], FP8, isOutput=False)
    wh0dt = nc.declare_dram_parameter("wh0dt", [128, KC, G3], FP8, isOutput=False)
    wi1dt = nc.declare_dram_parameter("wi1dt", [128, KC, G3], FP8, isOutput=False)
    wh1dt = nc.declare_dram_parameter("wh1dt", [128, KC, G3], FP8, isOutput=False)
    biasA = nc.declare_dram_parameter("biasA", [128, NCH], F32, isOutput=False)
    b2rz = nc.declare_dram_parameter("b2rz", [1, 2 * H], BF16, isOutput=False)
    b2n = nc.declare_dram_parameter("b2n", [1, H], BF16, isOutput=False)
    bhn1 = nc.declare_dram_parameter("bhn1", [1, H], BF16, isOutput=False)
    bhn2 = nc.declare_dram_parameter("bhn2", [1, H], BF16, isOutput=False)
    h1t0 = nc.declare_dram_parameter("h1t0", [128, KC, R], BF16, isOutput=False)
    h1t08 = nc.declare_dram_parameter("h1t08", [128, KC, R], FP8, isOutput=False)
    h2t08 = nc.declare_dram_parameter("h2t08", [128, KC, R], FP8, isOutput=False)
    h1t0d8 = nc.declare_dram_parameter("h1t0d8", [128, KC, R], FP8, isOutput=False)
    h2t0d8 = nc.declare_dram_parameter("h2t0d8", [128, KC, R], FP8, isOutput=False)
    h2t0 = nc.declare_dram_parameter("h2t0", [128, KC, R], BF16, isOutput=False)
    outp = nc.declare_dram_parameter("out", [OBLK, 128, KC, 8, R], BF16, isOutput=True)

    # gi1, per-step sliceable: [p, mc, sj8, r]; one tensor per 8-step
    # block so each step-loop prefetch only depends on its own A block.
    gi1b = [nc.dram_tensor(f"gi1b{bb}", [128, NCH, 8, R], BF16)
            for bb in range(OBLK)]

    with tile.TileContext(nc) as tc:
        with (
            tc.tile_pool(name="wpool", bufs=1) as wp,
        ):
            # Recurrence weights: tiles allocated here; their DMAs are
            # interleaved with phase A's input loads below so the x/wi0
            # chunks phase A needs first reach the DMA engines first.
            wh0s = wp.tile([128, KC, G3], FP8, name="wh0s")
            wi1s = wp.tile([128, KC, G3], FP8, name="wi1s")
            wh1s = wp.tile([128, KC, G3], FP8, name="wh1s")
            wh0d = wp.tile([128, KC, G3], FP8, name="wh0d")
            wi1d = wp.tile([128, KC, G3], FP8, name="wi1d")
            wh1d = wp.tile([128, KC, G3], FP8, name="wh1d")

            # ---------------- Phase A: gi1 = wi0 @ x.T + biases ----------------
            with (
                tc.tile_pool(name="axp", bufs=1) as axp,
                tc.tile_pool(name="axcp", bufs=2) as axcp,
                tc.tile_pool(name="apsp", bufs=6, space="PSUM") as apsp,
                tc.tile_pool(name="aevp", bufs=12) as aevp,
            ):
                # wi0 in quarters (separate tiles -> finer DMA deps, so the
                # first matmuls only wait on quarter 0).
                wi0q = [axp.tile([128, KC, G3 // 4], BF16, name=f"wi0q{q}")
                        for q in range(4)]
                biasAs = axp.tile([128, NCH], F32, name="biasAs")
                # SP queue carries only the no-wait input loads, in the order
                # A consumes them; gi1b writes ride the ACT queue behind the
                # ev activations that produce them; the big recurrence-weight
                # loads join the ACT queue mid-A to fill DMA idle time.
                xc0 = axcp.tile([128, KC, 256], BF16, tag="xc", name="xc0")
                nc.sync.dma_start(out=xc0, in_=xt[:, 0])
                nc.sync.dma_start(out=wi0q[0], in_=wi0t[:, 0])
                nc.sync.dma_start(out=biasAs, in_=biasA[:])
                for q in range(1, 4):
                    nc.sync.dma_start(out=wi0q[q], in_=wi0t[:, q])
                for bb in range(OBLK):
                    if bb == 0:
                        xc = xc0
                    else:
                        xc = axcp.tile([128, KC, 256], BF16, tag="xc",
                                       name=f"xc{bb}")
                        nc.sync.dma_start(out=xc, in_=xt[:, bb])
                    if bb == min(2, OBLK - 1):
                        # Quarter-sized weight loads, each pinned (via
                        # wait_until) into a staggered slot so they fill DMA
                        # idle time without displacing A's input loads or
                        # blocking the gi1b write stream for 17us at a time.
                        for wqi, wdst, wsrc in (
                                [(i, wh0s, wh0t) for i in range(4)]
                                + [(4 + i, wi1s, wi1t) for i in range(4)]
                                + [(8 + i, wh1s, wh1t) for i in range(4)]
                                + [(12 + i, wh0d, wh0dt) for i in range(4)]
                                + [(16 + i, wi1d, wi1dt) for i in range(4)]
                                + [(20 + i, wh1d, wh1dt) for i in range(4)]):
                            q = wqi % 4
                            cols = slice(q * (G3 // 4), (q + 1) * (G3 // 4))
                            with tc.tile_wait_until(0.020 + 0.0045 * wqi):
                                nc.scalar.dma_start(
                                    out=wdst[:, :, cols], in_=wsrc[:, :, cols])
                    for mc in range(NCH):
                        ps = apsp.tile([128, 256], F32, tag="aps", name=f"aps_{mc}_{bb}")
                        for k in range(KC):
                            q, qo = divmod(mc, 6)
                            nc.tensor.matmul(
                                ps, wi0q[q][:, k, qo * 128:(qo + 1) * 128],
                                xc[:, k, :],
                                start=(k == 0), stop=(k == KC - 1))
                        ev = aevp.tile([128, 8, R], BF16, tag="aev", name=f"aev_{mc}_{bb}")
                        nc.vector.tensor_scalar(
                            ev, ps.rearrange("p (s r) -> p s r", s=8),
                            SCL, biasAs[:, mc:mc + 1],
                            mybir.AluOpType.mult, mybir.AluOpType.add)
                        nc.scalar.dma_start(out=gi1b[bb][:, mc, :, :], in_=ev)

            # ---------------- Fused step loop (B + C + D) ----------------
            with (
                tc.tile_pool(name="cpool", bufs=1) as cp,
                tc.tile_pool(name="gip", bufs=2) as gip,
                tc.tile_pool(name="obp", bufs=2) as obp,
                tc.tile_pool(name="h1p", bufs=4) as h1p,
                tc.tile_pool(name="h8p", bufs=4) as h8p,
                tc.tile_pool(name="gtp", bufs=3) as gtp,
                tc.tile_pool(name="prz1", bufs=2, space="PSUM") as prz1,
                tc.tile_pool(name="pn1", bufs=1, space="PSUM") as pn1,
                tc.tile_pool(name="prz2", bufs=2, space="PSUM") as prz2,
                tc.tile_pool(name="pn2", bufs=1, space="PSUM") as pn2,
                tc.tile_pool(name="pgn2", bufs=2, space="PSUM") as pgn2,
            ):
                identf = cp.tile([128, 128], F32, name="identf")
                make_identity(nc, identf)
                ident = cp.tile([128, 128], BF16, name="ident")
                nc.vector.tensor_copy(ident, identf)
                ones = cp.tile([1, R], BF16, name="ones")
                nc.vector.memset(ones, 1.0)
                bhn1s = cp.tile([1, H], BF16, name="bhn1s")
                nc.sync.dma_start(out=bhn1s, in_=bhn1[:])
                bhn2s = cp.tile([1, H], BF16, name="bhn2s")
                nc.sync.dma_start(out=bhn2s, in_=bhn2[:])
                b2rzs = cp.tile([1, 2 * H], BF16, name="b2rzs")
                nc.sync.dma_start(out=b2rzs, in_=b2rz[:])
                b2ns = cp.tile([1, H], BF16, name="b2ns")
                nc.sync.dma_start(out=b2ns, in_=b2n[:])
                h1t0s = cp.tile([128, KC, R], BF16, name="h1t0s")
                nc.sync.dma_start(out=h1t0s, in_=h1t0[:])
                h2t0s = cp.tile([128, KC, R], BF16, name="h2t0s")
                nc.sync.dma_start(out=h2t0s, in_=h2t0[:])
                h1t08s = cp.tile([128, KC, R], FP8, name="h1t08s")
                nc.sync.dma_start(out=h1t08s, in_=h1t08[:])
                h2t08s = cp.tile([128, KC, R], FP8, name="h2t08s")
                nc.sync.dma_start(out=h2t08s, in_=h2t08[:])
                h1t0d8s = cp.tile([128, KC, R], FP8, name="h1t0d8s")
                nc.sync.dma_start(out=h1t0d8s, in_=h1t0d8[:])
                h2t0d8s = cp.tile([128, KC, R], FP8, name="h2t0d8s")
                nc.sync.dma_start(out=h2t0d8s, in_=h2t0d8[:])

                def prefetch(bb):
                    # gpsimd (SWDGE) queue: independent of the SP queue that
                    # carries phase A's gi1b writes, so each prefetch fires as
                    # soon as its own 8-step block's writes complete.
                    g = gip.tile([128, NCH, 8, R], BF16, tag="gi", name=f"gi_{bb}")
                    nc.gpsimd.dma_start(out=g, in_=gi1b[bb][:])
                    return g

                def gates(s, hh, rzp, npp, gin_ap, hold, hout_ap, ln, hout8_ap,
                          hd8_ap):
                    """One half (4 feature chunks) of GRU gate math.

                    PSUM gate pre-activations and gi1 carry an extra SCL
                    factor (fp8 weights are pre-scaled); the sigmoid/tanh
                    fold it back out via their input-scale parameter.
                    """
                    c0 = 4 * hh
                    rzs = gtp.tile([128, 2, 4, R], BF16, tag=f"rzs{ln}",
                                   name=f"rzs{ln}_{s}_{hh}")
                    nc.scalar.activation(rzs, rzp[:, :, c0:c0 + 4, :], AF.Sigmoid,
                                         scale=1.0 / SCL)
                    t1 = gtp.tile([128, 4, R], BF16, tag=f"t1{ln}",
                                  name=f"t1{ln}_{s}_{hh}")
                    nc.vector.tensor_mul(t1, rzs[:, 0], npp[:, c0:c0 + 4, :])
                    npre = gtp.tile([128, 4, R], BF16, tag=f"npre{ln}",
                                    name=f"npre{ln}_{s}_{hh}")
                    nc.vector.tensor_add(npre, t1, gin_ap)
                    nn = gtp.tile([128, 4, R], BF16, tag=f"nn{ln}",
                                  name=f"nn{ln}_{s}_{hh}")
                    nc.scalar.activation(nn, npre, AF.Tanh, scale=1.0 / SCL)
                    dd = gtp.tile([128, 4, R], BF16, tag=f"dd{ln}",
                                  name=f"dd{ln}_{s}_{hh}")
                    nc.vector.tensor_sub(dd, hold[:, c0:c0 + 4, :], nn)
                    t2 = gtp.tile([128, 4, R], BF16, tag=f"t2{ln}",
                                  name=f"t2{ln}_{s}_{hh}")
                    nc.vector.tensor_mul(t2, rzs[:, 1], dd)
                    nc.vector.tensor_add(hout_ap, nn, t2)
                    nc.vector.tensor_copy(hout8_ap, hout_ap)
                    nc.vector.tensor_sub(hd8_ap, hout_ap, hout8_ap)

                gtile = prefetch(0)
                gnext = None
                h1prev = h1t0s
                h2prev = h2t0s
                h1p8 = h1t08s
                h2p8 = h2t08s
                h1pd8 = h1t0d8s
                h2pd8 = h2t0d8s
                ob = None
                for s in range(steps):
                    bb, sj = divmod(s, 8)
                    if sj == 0:
                        if bb > 0:
                            gtile = gnext
                        ob = obp.tile([128, KC, 8, R], BF16, tag="ob",
                                      name=f"ob_{bb}")

                    # ---- layer 1: gh1 (+gi1 rz, +bhn1), gates by halves ----
                    # PSUM zero-region rule: exactly ONE start (first matmul
                    # into the tile) and ONE stop (last matmul into the tile)
                    # per step -- a second start=True would mark the whole 2KB
                    # zero region pending-zero and destroy sibling partials.
                    rz1 = prz1.tile([128, 2, KC, R], F32, tag="rz1", name=f"rz1_{s}")
                    n1 = pn1.tile([128, KC, R], F32, tag="n1", name=f"n1_{s}")
                    h1new = h1p.tile([128, KC, R], BF16, tag="h1", name=f"h1_{s}")
                    h1new8 = h8p.tile([128, KC, R], FP8, tag="h18",
                                      name=f"h18_{s}")
                    h2new8 = h8p.tile([128, KC, R], FP8, tag="h28",
                                      name=f"h28_{s}")
                    h1newd8 = h8p.tile([128, KC, R], FP8, tag="h1d8",
                                       name=f"h1d8_{s}")
                    h2newd8 = h8p.tile([128, KC, R], FP8, tag="h2d8",
                                       name=f"h2d8_{s}")
                    for hh in range(2):
                        c0 = 4 * hh
                        l1sets = ((wh0s, h1p8), (wh0s, h1pd8), (wh0d, h1p8))
                        for g in range(2):
                            for c in range(c0, c0 + 4):
                                n = g * 8 + c
                                for si, (wt, ht) in enumerate(l1sets):
                                    for jp in range(4):
                                        nc.tensor.matmul(
                                            rz1[:, g, c, :],
                                            wt[:, 2 * jp:2 * jp + 2,
                                               n * 128:(n + 1) * 128],
                                            ht[:, 2 * jp:2 * jp + 2, :],
                                            perf_mode=DR,
                                            start=(hh == 0 and g == 0 and c == 0
                                                   and si == 0 and jp == 0),
                                            stop=False)
                                nc.tensor.matmul(
                                    rz1[:, g, c, :], ident, gtile[:, n, sj, :],
                                    start=False,
                                    stop=(hh == 1 and g == 1 and c == 7))
                        for c in range(c0, c0 + 4):
                            n = 16 + c
                            for si, (wt, ht) in enumerate(l1sets):
                                for jp in range(4):
                                    nc.tensor.matmul(
                                        n1[:, c, :],
                                        wt[:, 2 * jp:2 * jp + 2,
                                           n * 128:(n + 1) * 128],
                                        ht[:, 2 * jp:2 * jp + 2, :],
                                        perf_mode=DR,
                                        start=(hh == 0 and c == 0 and si == 0
                                               and jp == 0),
                                        stop=False)
                            nc.tensor.matmul(
                                n1[:, c, :], bhn1s[0:1, c * 128:(c + 1) * 128],
                                ones, start=False, stop=(hh == 1 and c == 7))
                        gates(s, hh, rz1, n1, gtile[:, 16 + c0:16 + c0 + 4, sj, :],
                              h1prev, h1new[:, c0:c0 + 4, :], "a",
                              h1new8[:, c0:c0 + 4, :],
                              h1newd8[:, c0:c0 + 4, :])

                    # ---- layer 2 recurrent gh2 (k-half split; h2prev) ----
                    rz2 = prz2.tile([128, 2, KC, R], F32, tag="rz2", name=f"rz2_{s}")
                    n2 = pn2.tile([128, KC, R], F32, tag="n2", name=f"n2_{s}")
                    gn2 = pgn2.tile([128, KC, R], F32, tag="gn2", name=f"gn2_{s}")
                    l2sets = ((wh1s, h2p8), (wh1s, h2pd8), (wh1d, h2p8))
                    for kh in range(2):
                        for g in range(2):
                            for c in range(KC):
                                n = g * 8 + c
                                for si, (wt, ht) in enumerate(l2sets):
                                    for jp in range(2 * kh, 2 * kh + 2):
                                        nc.tensor.matmul(
                                            rz2[:, g, c, :],
                                            wt[:, 2 * jp:2 * jp + 2,
                                               n * 128:(n + 1) * 128],
                                            ht[:, 2 * jp:2 * jp + 2, :],
                                            perf_mode=DR,
                                            start=(kh == 0 and g == 0 and c == 0
                                                   and si == 0 and jp == 0),
                                            stop=False)
                        for c in range(KC):
                            n = 16 + c
                            for si, (wt, ht) in enumerate(l2sets):
                                for jp in range(2 * kh, 2 * kh + 2):
                                    nc.tensor.matmul(
                                        n2[:, c, :],
                                        wt[:, 2 * jp:2 * jp + 2,
                                           n * 128:(n + 1) * 128],
                                        ht[:, 2 * jp:2 * jp + 2, :],
                                        perf_mode=DR,
                                        start=(kh == 0 and c == 0 and si == 0
                                               and jp == 0),
                                        stop=False)
                    for g in range(2):
                        for c in range(KC):
                            n = g * 8 + c
                            nc.tensor.matmul(
                                rz2[:, g, c, :], b2rzs[0:1, n * 128:(n + 1) * 128],
                                ones, start=False, stop=False)
                    for c in range(KC):
                        nc.tensor.matmul(
                            n2[:, c, :], bhn2s[0:1, c * 128:(c + 1) * 128],
                            ones, start=False, stop=(c == 7))

                    # ---- gi2 = wi1 @ h1new, accumulated into rz2 / gn2 ----
                    # gn2's group opener is the first gi2 matmul (which waits
                    # on h1new anyway); a dep-free opener like the b2n bias
                    # matmul gets hoisted by the scheduler and then HOL-blocks
                    # the PE queue on its PSUM WAR wait.
                    gisets = ((wi1s, h1new8), (wi1s, h1newd8), (wi1d, h1new8))
                    for kh in range(2):
                        for g in range(2):
                            for c in range(KC):
                                n = g * 8 + c
                                for si, (wt, ht) in enumerate(gisets):
                                    for jp in range(2 * kh, 2 * kh + 2):
                                        nc.tensor.matmul(
                                            rz2[:, g, c, :],
                                            wt[:, 2 * jp:2 * jp + 2,
                                               n * 128:(n + 1) * 128],
                                            ht[:, 2 * jp:2 * jp + 2, :],
                                            perf_mode=DR,
                                            start=False,
                                            stop=(kh == 1 and g == 1 and c == 7
                                                  and si == 2 and jp == 3))
                        for c in range(KC):
                            n = 16 + c
                            for si, (wt, ht) in enumerate(gisets):
                                for jp in range(2 * kh, 2 * kh + 2):
                                    nc.tensor.matmul(
                                        gn2[:, c, :],
                                        wt[:, 2 * jp:2 * jp + 2,
                                           n * 128:(n + 1) * 128],
                                        ht[:, 2 * jp:2 * jp + 2, :],
                                        perf_mode=DR,
                                        start=(kh == 0 and c == 0 and si == 0
                                               and jp == 0),
                                        stop=False)
                    for c in range(KC):
                        nc.tensor.matmul(
                            gn2[:, c, :], b2ns[0:1, c * 128:(c + 1) * 128],
                            ones, start=False, stop=(c == 7))

                    if sj == 0 and bb + 1 < OBLK:
                        gnext = prefetch(bb + 1)

                    # ---- layer 2 gates -> output staging ----
                    for hh in range(2):
                        gates(s, hh, rz2, n2, gn2[:, 4 * hh:4 * hh + 4, :],
                              h2prev, ob[:, 4 * hh:4 * hh + 4, sj, :], "b",
                              h2new8[:, 4 * hh:4 * hh + 4, :],
                              h2newd8[:, 4 * hh:4 * hh + 4, :])

                    h2prev = ob[:, :, sj, :]
                    h1p8 = h1new8
                    h2p8 = h2new8
                    h1pd8 = h1newd8
                    h2pd8 = h2newd8
                    if debug:
                        nc.sync.dma_start(out=dbgh1[s], in_=h1new)
                    h1prev = h1new
                    if sj == 7:
                        nc.sync.dma_start(out=outp[bb], in_=ob)
                if debug:
                    dcp = gip.tile([128, NCH, 8, R], BF16, tag="gi", name="dcp")
                    for bb in range(OBLK):
                        nc.sync.dma_start(out=dcp, in_=gi1b[bb][:])
                        nc.sync.dma_start(
                            out=dbgi[:, :, bb // 2,
                                     (bb % 2) * 8:(bb % 2) * 8 + 8, :],
                            in_=dcp)

    nc.finalize()
    return nc


def ode_traj(w1, b1, w2, b2, w3, b3):
    """RK4 trajectory of the ODE, mirroring the reference exactly (fp32)."""
    w1t = w1.T.astype(np.float32)
    w2t = w2.T.astype(np.float32)
    w3t = w3.T.astype(np.float32)

    def f(h):
        a = np.tanh(h @ w1t + b1)
        a = np.tanh(a @ w2t + b2)
        return a @ w3t + b3

    dt = np.float32((1.0 / NSEG) / SUB)
    h = np.zeros((2, H), np.float32)
    traj = []
    for _ in range(NSEG):
        for _ in range(SUB):
            k1 = f(h)
            k2 = f(h + np.float32(0.5) * dt * k1)
            k3 = f(h + np.float32(0.5) * dt * k2)
            k4 = f(h + dt * k3)
            h = h + (dt / np.float32(6.0)) * (k1 + np.float32(2.0) * k2
                                              + np.float32(2.0) * k3 + k4)
        traj.append(h.copy())
    return np.stack(traj)  # (NSEG, 2, H)


def _wt(w):
    """[G3, H] weight -> feature-major [128, KC, G3] fp16."""
    return np.ascontiguousarray(
        w.T.reshape(KC, 128, G3).transpose(1, 0, 2)).astype(NP_BF16)


def _wt8(w):
    """[G3, H] weight -> feature-major [128, KC, G3] fp8, pre-scaled by SCL."""
    return np.ascontiguousarray(
        (w.T * SCL).reshape(KC, 128, G3).transpose(1, 0, 2)).astype(NP_FP8)


def _wtd8(w):
    """fp8 quantization residual of _wt8(w), itself in fp8."""
    w16 = np.ascontiguousarray(
        (w.T * SCL).reshape(KC, 128, G3).transpose(1, 0, 2)).astype(np.float32)
    return (w16 - w16.astype(NP_FP8).astype(np.float32)).astype(NP_FP8)


def make_in_maps(x, w1, b1, w2, b2, w3, b3, wi0, wh0, bi0, bh0,
                 wi1, wh1, bi1, bh1, steps=S, cores=NCORES):
    traj = ode_traj(w1, b1, w2, b2, w3, b3)

    # xt[p, bb, k, sj*R + r] = x[bb*8+sj, r, k*128+p]
    xp = np.zeros((128, KC, steps, R), np.float32)
    xp[:, :, :, :T] = x[:steps].reshape(steps, T, KC, 128).transpose(3, 2, 0, 1)
    xt = np.ascontiguousarray(
        xp.reshape(128, KC, steps // 8, 8 * R).transpose(0, 2, 1, 3)
    ).astype(NP_BF16)

    biasA = np.concatenate([bi0[:2 * H] + bh0[:2 * H], bi0[2 * H:]])
    wi0q = np.ascontiguousarray(
        _wt(wi0).reshape(128, KC, 4, G3 // 4).transpose(0, 2, 1, 3))
    shared = {
        "xt": xt,
        "wi0t": wi0q, "wh0t": _wt8(wh0), "wh0dt": _wtd8(wh0),
        "wi1t": _wt8(wi1), "wh1t": _wt8(wh1),
        "wi1dt": _wtd8(wi1), "wh1dt": _wtd8(wh1),
        "biasA": np.ascontiguousarray(
            SCL * biasA.reshape(NCH, 128).T).astype(np.float32),
        "b2rz": (SCL * (bi1[:2 * H] + bh1[:2 * H])).reshape(1, 2 * H).astype(NP_BF16),
        "b2n": (SCL * bi1[2 * H:]).reshape(1, H).astype(NP_BF16),
        "bhn1": (SCL * bh0[2 * H:]).reshape(1, H).astype(NP_BF16),
        "bhn2": (SCL * bh1[2 * H:]).reshape(1, H).astype(NP_BF16),
    }
    in_maps = []
    for i in range(cores):
        m = dict(shared)
        for nm, vec in (("h1t0", traj[i, 0]), ("h2t0", traj[i, 1])):
            ht = np.ascontiguousarray(np.broadcast_to(
                vec.reshape(KC, 128).T[:, :, None], (128, KC, R)))
            m[nm] = ht.astype(NP_BF16)
            m[nm + "8"] = ht.astype(NP_FP8)
            m[nm + "d8"] = (ht - ht.astype(NP_FP8).astype(np.float32)
                            ).astype(NP_FP8)
        in_maps.append(m)
    return in_maps


_NC_CACHE = {}


def _get_nc(steps):
    if steps not in _NC_CACHE:
        _NC_CACHE[steps] = build_nc(steps)
    return _NC_CACHE[steps]


def run_cores(inputs, steps=S, cores=NCORES, **run_kwargs):
    in_maps = make_in_maps(steps=steps, cores=cores, **inputs)
    nc = _get_nc(steps)
    return run_bass_kernel_spmd(nc, in_maps, core_ids=list(range(cores)),
                                **run_kwargs)


def kernel(x, w1, b1, w2, b2, w3, b3, wi0, wh0, bi0, bh0,
           wi1, wh1, bi1, bh1):
    args = dict(x=x, w1=w1, b1=b1, w2=w2, b2=b2, w3=w3, b3=b3,
                wi0=wi0, wh0=wh0, bi0=bi0, bh0=bh0,
                wi1=wi1, wh1=wh1, bi1=bi1, bh1=bh1)
    args = {k: np.asarray(v, np.float32) for k, v in args.items()}
    res = run_cores(args, steps=S, cores=NCORES)
    full = np.empty((S, T * NCORES, H), np.float32)
    for i in range(NCORES):
        o = np.asarray(res.results[i]["out"]).astype(np.float32)
        # [ob, p, k, sj, r] -> [s, r, feat]
        o = o.transpose(0, 3, 4, 2, 1).reshape(S, R, H)
        full[:, i::NCORES, :] = o[:, :T, :]
    return full


# revision 55
# speedup vs baseline: 3.4413x; 1.0036x over previous
"""ODE-RNN Trainium2 kernel (v2: feature-major bf16 formulation).

Problem: out[b, t*8+i, :] = 2-layer GRU (H=1024) run over the batch dim
(64 steps) of sequence t (30 sequences), with initial hiddens taken from an
RK4-integrated ODE trajectory (8 grid points, shared across all runs).

Strategy (8 NeuronCores, pure data-parallel, no collectives):
  - ODE trajectory on host (tiny, exactly mirrors reference math).
  - Core i handles the 30 GRU runs with init traj[i]; R=32 padded runs.
  - Everything on-device is kept FEATURE-MAJOR (transposed): state tensors
    live as [128 partitions = feature-within-chunk, KC=8 chunks, R runs].
    The recurrent matmuls then use the (resident, bf16) weights as the
    128x128 PE-stationary operand and stream the small [128, R] state as
    the moving operand: cost-model rows per step drop 4x vs streaming the
    weights, and no PE transposes are needed anywhere.
  - Phase A: gi1 = wi0 @ x.T + bias as a dense GEMM, output to DRAM in a
    per-step-sliceable layout (bf16).
  - Fused step loop (phases B+C+D): per step s, layer-1 gates from
    (gi1[s], h1[s-1]); gi2 = wi1 @ h1[s] accumulated directly into the
    layer-2 gate PSUM; layer-2 gates -> h2[s] -> output staging buffer.
    All biases are accumulated on the PE (identity / ones-row matmuls),
    so the vector/scalar engines only do the nonlinear gate math.
  - Emission is ordered so the PE never waits on the gate-math chains:
    layer-1 gate math overlaps layer-2 recurrent matmuls, and the fresh
    states are consumed k-half by k-half.
"""

import numpy as np

try:
    import concourse.bass as bass  # noqa: F401
except ImportError:  # pragma: no cover - fallback for bare environments
    import sys
    sys.path.insert(0, "/opt/trn_rl_repo")
    import concourse.bass as bass  # noqa: F401

import concourse.mybir as mybir
import concourse.tile as tile
from concourse import bacc
from concourse.bass_utils import run_bass_kernel_spmd
from concourse.masks import make_identity

F32 = mybir.dt.float32
FP8 = mybir.dt.float8e4
NP_FP8 = mybir.dt.np(mybir.dt.float8e4)
DR = mybir.MatmulPerfMode.DoubleRow
SCL = 16.0  # fp8 weight pre-scale; folded back via ACT scale=1/SCL
BF16 = mybir.dt.float16  # fp16: same PE cost as bf16, more mantissa
NP_BF16 = mybir.dt.np(mybir.dt.float16)
AF = mybir.ActivationFunctionType

H = 1024        # hidden size
KC = H // 128   # feature chunks (8)
G3 = 3 * H      # gate width
NCH = G3 // 128  # gate feature chunks (24)
T = 30          # sequences
R = 32          # padded runs per core (30 real + 2 pad)
S = 64          # steps (batch dim acts as sequence length)
NSEG = 8
SUB = 4
NCORES = 8


def build_nc(steps=S, debug=False):
    """Build the per-core Bass module (same program on all 8 cores)."""
    NB = steps // 16          # 512-column blocks in phase A
    OBLK = steps // 8         # 8-step output/gi blocks
    nc = bacc.Bacc()
    if debug:
        dbgi = nc.declare_dram_parameter(
            "dbgi", [128, NCH, NB, 16, R], BF16, isOutput=True)
        dbgh1 = nc.declare_dram_parameter(
            "dbgh1", [steps, 128, KC, R], BF16, isOutput=True)

    xt = nc.declare_dram_parameter("xt", [128, steps // 8, KC, 256], FP8,
                                   isOutput=False)
    xtd = nc.declare_dram_parameter("xtd", [128, steps // 8, KC, 256], FP8,
                                    isOutput=False)
    wi0t = nc.declare_dram_parameter("wi0t", [128, 4, KC, G3 // 4], FP8,
                                     isOutput=False)
    wi0dt = nc.declare_dram_parameter("wi0dt", [128, 4, KC, G3 // 4], FP8,
                                      isOutput=False)
    wh0t = nc.declare_dram_parameter("wh0t", [128, KC, G3], FP8, isOutput=False)
    wi1t = nc.declare_dram_parameter("wi1t", [128, KC, G3], FP8, isOutput=False)
    wh1t = nc.declare_dram_parameter("wh1t", [128, KC, G3], FP8, isOutput=False)
    wh0dt = nc.declare_dram_parameter("wh0dt", [128, KC, G3], FP8, isOutput=False)
    wi1dt = nc.declare_dram_parameter("wi1dt", [128, KC, G3], FP8, isOutput=False)
    wh1dt = nc.declare_dram_parameter("wh1dt", [128, KC, G3], FP8, isOutput=False)
    biasA = nc.declare_dram_parameter("biasA", [128, NCH], F32, isOutput=False)
    b2rz = nc.declare_dram_parameter("b2rz", [1, 2 * H], BF16, isOutput=False)
    b2n = nc.declare_dram_parameter("b2n", [1, H], BF16, isOutput=False)
    bhn1 = nc.declare_dram_parameter("bhn1", [1, H], BF16, isOutput=False)
    bhn2 = nc.declare_dram_parameter("bhn2", [1, H], BF16, isOutput=False)
    h1t0 = nc.declare_dram_parameter("h1t0", [128, KC, R], BF16, isOutput=False)
    h1t08 = nc.declare_dram_parameter("h1t08", [128, KC, R], FP8, isOutput=False)
    h2t08 = nc.declare_dram_parameter("h2t08", [128, KC, R], FP8, isOutput=False)
    h1t0d8 = nc.declare_dram_parameter("h1t0d8", [128, KC, R], FP8, isOutput=False)
    h2t0d8 = nc.declare_dram_parameter("h2t0d8", [128, KC, R], FP8, isOutput=False)
    h2t0 = nc.declare_dram_parameter("h2t0", [128, KC, R], BF16, isOutput=False)
    outp = nc.declare_dram_parameter("out", [OBLK, 128, KC, 8, R], BF16, isOutput=True)

    # gi1, per-step sliceable: [p, mc, sj8, r]; one tensor per 8-step
    # block so each step-loop prefetch only depends on its own A block.
    gi1b = [nc.dram_tensor(f"gi1b{bb}", [128, NCH, 8, R], BF16)
            for bb in range(OBLK)]

    with tile.TileContext(nc) as tc:
        with (
            tc.tile_pool(name="wpool", bufs=1) as wp,
        ):
            # Recurrence weights: tiles allocated here; their DMAs are
            # interleaved with phase A's input loads below so the x/wi0
            # chunks phase A needs first reach the DMA engines first.
            wh0s = wp.tile([128, KC, G3], FP8, name="wh0s")
            wi1s = wp.tile([128, KC, G3], FP8, name="wi1s")
            wh1s = wp.tile([128, KC, G3], FP8, name="wh1s")
            wh0d = wp.tile([128, KC, G3], FP8, name="wh0d")
            wi1d = wp.tile([128, KC, G3], FP8, name="wi1d")
            wh1d = wp.tile([128, KC, G3], FP8, name="wh1d")

            # ---------------- Phase A: gi1 = wi0 @ x.T + biases ----------------
            with (
                tc.tile_pool(name="axp", bufs=1) as axp,
                tc.tile_pool(name="axcp", bufs=2) as axcp,
                tc.tile_pool(name="apsp", bufs=6, space="PSUM") as apsp,
                tc.tile_pool(name="aevp", bufs=12) as aevp,
            ):
                # wi0 in quarters (separate tiles -> finer DMA deps, so the
                # first matmuls only wait on quarter 0).
                wi0q = [axp.tile([128, KC, G3 // 4], FP8, name=f"wi0q{q}")
                        for q in range(4)]
                wi0qd = [axp.tile([128, KC, G3 // 4], FP8, name=f"wi0qd{q}")
                         for q in range(4)]
                biasAs = axp.tile([128, NCH], F32, name="biasAs")
                # SP queue carries only the no-wait input loads, in the order
                # A consumes them; gi1b writes ride the ACT queue behind the
                # ev activations that produce them; the big recurrence-weight
                # loads join the ACT queue mid-A to fill DMA idle time.
                xc0 = axcp.tile([128, KC, 256], FP8, tag="xc", name="xc0")
                nc.sync.dma_start(out=xc0, in_=xt[:, 0])
                xcd0 = axcp.tile([128, KC, 256], FP8, tag="xcd", name="xcd0")
                nc.sync.dma_start(out=xcd0, in_=xtd[:, 0])
                nc.sync.dma_start(out=wi0q[0], in_=wi0t[:, 0])
                nc.sync.dma_start(out=wi0qd[0], in_=wi0dt[:, 0])
                nc.sync.dma_start(out=biasAs, in_=biasA[:])
                for q in range(1, 4):
                    nc.sync.dma_start(out=wi0q[q], in_=wi0t[:, q])
                    nc.sync.dma_start(out=wi0qd[q], in_=wi0dt[:, q])
                for bb in range(OBLK):
                    if bb == 0:
                        xc, xcd = xc0, xcd0
                    else:
                        xc = axcp.tile([128, KC, 256], FP8, tag="xc",
                                       name=f"xc{bb}")
                        nc.sync.dma_start(out=xc, in_=xt[:, bb])
                        xcd = axcp.tile([128, KC, 256], FP8, tag="xcd",
                                        name=f"xcd{bb}")
                        nc.sync.dma_start(out=xcd, in_=xtd[:, bb])
                    if bb == min(2, OBLK - 1):
                        # Quarter-sized weight loads, each pinned (via
                        # wait_until) into a staggered slot so they fill DMA
                        # idle time without displacing A's input loads or
                        # blocking the gi1b write stream for 17us at a time.
                        for wqi, wdst, wsrc in (
                                [(i, wh0s, wh0t) for i in range(4)]
                                + [(4 + i, wi1s, wi1t) for i in range(4)]
                                + [(8 + i, wh1s, wh1t) for i in range(4)]
                                + [(12 + i, wh0d, wh0dt) for i in range(4)]
                                + [(16 + i, wi1d, wi1dt) for i in range(4)]
                                + [(20 + i, wh1d, wh1dt) for i in range(4)]):
                            q = wqi % 4
                            cols = slice(q * (G3 // 4), (q + 1) * (G3 // 4))
                            with tc.tile_wait_until(0.012 + 0.0022 * wqi):
                                nc.scalar.dma_start(
                                    out=wdst[:, :, cols], in_=wsrc[:, :, cols])
                    for mc in range(NCH):
                        q, qo = divmod(mc, 6)
                        ps = apsp.tile([128, 256], F32, tag="aps", name=f"aps_{mc}_{bb}")
                        asets = ((wi0q[q], xc), (wi0q[q], xcd), (wi0qd[q], xc))
                        for si, (wt, xt_) in enumerate(asets):
                            for jp in range(4):
                                nc.tensor.matmul(
                                    ps,
                                    wt[:, 2 * jp:2 * jp + 2,
                                       qo * 128:(qo + 1) * 128],
                                    xt_[:, 2 * jp:2 * jp + 2, :],
                                    perf_mode=DR,
                                    start=(si == 0 and jp == 0),
                                    stop=(si == 2 and jp == 3))
                        ev = aevp.tile([128, 8, R], BF16, tag="aev", name=f"aev_{mc}_{bb}")
                        nc.vector.tensor_scalar_add(
                            ev, ps.rearrange("p (s r) -> p s r", s=8),
                            biasAs[:, mc:mc + 1])
                        weng = nc.scalar if mc % 2 == 0 else nc.sync
                        weng.dma_start(out=gi1b[bb][:, mc, :, :], in_=ev)

            # ---------------- Fused step loop (B + C + D) ----------------
            with (
                tc.tile_pool(name="cpool", bufs=1) as cp,
                tc.tile_pool(name="gip", bufs=2) as gip,
                tc.tile_pool(name="obp", bufs=2) as obp,
                tc.tile_pool(name="h1p", bufs=4) as h1p,
                tc.tile_pool(name="h8p", bufs=4) as h8p,
                tc.tile_pool(name="gtp", bufs=3) as gtp,
                tc.tile_pool(name="prz1", bufs=1, space="PSUM") as prz1,
                tc.tile_pool(name="pn1", bufs=1, space="PSUM") as pn1,
                tc.tile_pool(name="prz2", bufs=2, space="PSUM") as prz2,
                tc.tile_pool(name="pn2", bufs=2, space="PSUM") as pn2,
                tc.tile_pool(name="pgn2", bufs=2, space="PSUM") as pgn2,
            ):
                identf = cp.tile([128, 128], F32, name="identf")
                make_identity(nc, identf)
                ident = cp.tile([128, 128], BF16, name="ident")
                nc.vector.tensor_copy(ident, identf)
                ones = cp.tile([1, R], BF16, name="ones")
                nc.vector.memset(ones, 1.0)
                bhn1s = cp.tile([1, H], BF16, name="bhn1s")
                nc.sync.dma_start(out=bhn1s, in_=bhn1[:])
                bhn2s = cp.tile([1, H], BF16, name="bhn2s")
                nc.sync.dma_start(out=bhn2s, in_=bhn2[:])
                b2rzs = cp.tile([1, 2 * H], BF16, name="b2rzs")
                nc.sync.dma_start(out=b2rzs, in_=b2rz[:])
                b2ns = cp.tile([1, H], BF16, name="b2ns")
                nc.sync.dma_start(out=b2ns, in_=b2n[:])
                h1t0s = cp.tile([128, KC, R], BF16, name="h1t0s")
                nc.sync.dma_start(out=h1t0s, in_=h1t0[:])
                h2t0s = cp.tile([128, KC, R], BF16, name="h2t0s")
                nc.sync.dma_start(out=h2t0s, in_=h2t0[:])
                h1t08s = cp.tile([128, KC, R], FP8, name="h1t08s")
                nc.sync.dma_start(out=h1t08s, in_=h1t08[:])
                h2t08s = cp.tile([128, KC, R], FP8, name="h2t08s")
                nc.sync.dma_start(out=h2t08s, in_=h2t08[:])
                h1t0d8s = cp.tile([128, KC, R], FP8, name="h1t0d8s")
                nc.sync.dma_start(out=h1t0d8s, in_=h1t0d8[:])
                h2t0d8s = cp.tile([128, KC, R], FP8, name="h2t0d8s")
                nc.sync.dma_start(out=h2t0d8s, in_=h2t0d8[:])

                def prefetch(bb):
                    # gpsimd (SWDGE) queue: independent of the SP queue that
                    # carries phase A's gi1b writes, so each prefetch fires as
                    # soon as its own 8-step block's writes complete.
                    g = gip.tile([128, NCH, 8, R], BF16, tag="gi", name=f"gi_{bb}")
                    nc.gpsimd.dma_start(out=g, in_=gi1b[bb][:])
                    return g

                def gates(s, hh, rzp, npp, gin_ap, hold, hout_ap, ln, hout8_ap,
                          hd8_ap):
                    """One half (4 feature chunks) of GRU gate math.

                    PSUM gate pre-activations and gi1 carry an extra SCL
                    factor (fp8 weights are pre-scaled); the sigmoid/tanh
                    fold it back out via their input-scale parameter.
                    """
                    c0 = 4 * hh
                    rzs = gtp.tile([128, 2, 4, R], BF16, tag=f"rzs{ln}",
                                   name=f"rzs{ln}_{s}_{hh}")
                    nc.scalar.activation(rzs, rzp[:, :, c0:c0 + 4, :], AF.Sigmoid,
                                         scale=1.0 / SCL)
                    t1 = gtp.tile([128, 4, R], BF16, tag=f"t1{ln}",
                                  name=f"t1{ln}_{s}_{hh}")
                    nc.vector.tensor_mul(t1, rzs[:, 0], npp[:, c0:c0 + 4, :])
                    npre = gtp.tile([128, 4, R], BF16, tag=f"npre{ln}",
                                    name=f"npre{ln}_{s}_{hh}")
                    nc.vector.tensor_add(npre, t1, gin_ap)
                    nn = gtp.tile([128, 4, R], BF16, tag=f"nn{ln}",
                                  name=f"nn{ln}_{s}_{hh}")
                    nc.scalar.activation(nn, npre, AF.Tanh, scale=1.0 / SCL)
                    dd = gtp.tile([128, 4, R], BF16, tag=f"dd{ln}",
                                  name=f"dd{ln}_{s}_{hh}")
                    nc.vector.tensor_sub(dd, hold[:, c0:c0 + 4, :], nn)
                    t2 = gtp.tile([128, 4, R], BF16, tag=f"t2{ln}",
                                  name=f"t2{ln}_{s}_{hh}")
                    nc.vector.tensor_mul(t2, rzs[:, 1], dd)
                    nc.vector.tensor_add(hout_ap, nn, t2)
                    nc.vector.tensor_copy(hout8_ap, hout_ap)
                    nc.vector.tensor_sub(hd8_ap, hout_ap, hout8_ap)

                gtile = prefetch(0)
                gnext = None
                h1prev = h1t0s
                h2prev = h2t0s
                h1p8 = h1t08s
                h2p8 = h2t08s
                h1pd8 = h1t0d8s
                h2pd8 = h2t0d8s

                ob = None
                # Software-pipeline rotation: iteration `it` FIRST finishes
                # step it-1's layer-2 gates (whose rz2/n2/gn2 PSUMs were
                # closed by gi2 at the end of iteration it-1), then runs
                # L1(it) / gh2(it) / gi2(it). With this emission order no
                # ACT op that transitively covers a fresh state write
                # precedes its PE consumers, so the framework's merged sem
                # waits cannot chain a step boundary to the previous step's
                # gate-math tail.
                pstate = None
                for it in range(steps + 1):
                    if pstate is not None:
                        rz2p, n2p, gn2p, h2n8p, h2nd8p, h2hold, p = pstate
                        pbb, psj = divmod(p, 8)
                        if psj == 0:
                            ob = obp.tile([128, KC, 8, R], BF16, tag="ob",
                                          name=f"ob_{pbb}")
                        for hh in range(2):
                            gates(p, hh, rz2p, n2p, gn2p[:, 4 * hh:4 * hh + 4, :],
                                  h2hold, ob[:, 4 * hh:4 * hh + 4, psj, :], "b",
                                  h2n8p[:, 4 * hh:4 * hh + 4, :],
                                  h2nd8p[:, 4 * hh:4 * hh + 4, :])
                        h2prev = ob[:, :, psj, :]
                        h2p8 = h2n8p
                        h2pd8 = h2nd8p
                        if psj == 7:
                            nc.sync.dma_start(out=outp[pbb], in_=ob)
                    if it == steps:
                        break
                    s = it
                    bb, sj = divmod(s, 8)
                    if sj == 0 and bb > 0:
                        gtile = gnext

                    # ---- layer 1: gh1 (+gi1 rz, +bhn1), gates by halves ----
                    # PSUM zero-region rule: exactly ONE start (first matmul
                    # into the tile) and ONE stop (last matmul into the tile)
                    # per step -- a second start=True would mark the whole 2KB
                    # zero region pending-zero and destroy sibling partials.
                    rz1 = prz1.tile([128, 2, KC, R], F32, tag="rz1", name=f"rz1_{s}")
                    n1 = pn1.tile([128, KC, R], F32, tag="n1", name=f"n1_{s}")
                    h1new = h1p.tile([128, KC, R], BF16, tag="h1", name=f"h1_{s}")
                    h1new8 = h8p.tile([128, KC, R], FP8, tag="h18",
                                      name=f"h18_{s}")
                    h2new8 = h8p.tile([128, KC, R], FP8, tag="h28",
                                      name=f"h28_{s}")
                    h1newd8 = h8p.tile([128, KC, R], FP8, tag="h1d8",
                                       name=f"h1d8_{s}")
                    h2newd8 = h8p.tile([128, KC, R], FP8, tag="h2d8",
                                       name=f"h2d8_{s}")
                    for hh in range(2):
                        c0 = 4 * hh
                        l1sets = ((wh0s, h1p8), (wh0s, h1pd8), (wh0d, h1p8))
                        for g in range(2):
                            for c in range(c0, c0 + 4):
                                n = g * 8 + c
                                for si, (wt, ht) in enumerate(l1sets):
                                    for jp in range(4):
                                        nc.tensor.matmul(
                                            rz1[:, g, c, :],
                                            wt[:, 2 * jp:2 * jp + 2,
                                               n * 128:(n + 1) * 128],
                                            ht[:, 2 * jp:2 * jp + 2, :],
                                            perf_mode=DR,
                                            start=(hh == 0 and g == 0 and c == 0
                                                   and si == 0 and jp == 0),
                                            stop=False)
                                nc.tensor.matmul(
                                    rz1[:, g, c, :], ident, gtile[:, n, sj, :],
                                    start=False,
                                    stop=(hh == 1 and g == 1 and c == 7))
                        for c in range(c0, c0 + 4):
                            n = 16 + c
                            for si, (wt, ht) in enumerate(l1sets):
                                for jp in range(4):
                                    nc.tensor.matmul(
                                        n1[:, c, :],
                                        wt[:, 2 * jp:2 * jp + 2,
                                           n * 128:(n + 1) * 128],
                                        ht[:, 2 * jp:2 * jp + 2, :],
                                        perf_mode=DR,
                                        start=(hh == 0 and c == 0 and si == 0
                                               and jp == 0),
                                        stop=False)
                            nc.tensor.matmul(
                                n1[:, c, :], bhn1s[0:1, c * 128:(c + 1) * 128],
                                ones, start=False, stop=(hh == 1 and c == 7))
                        gates(s, hh, rz1, n1, gtile[:, 16 + c0:16 + c0 + 4, sj, :],
                              h1prev, h1new[:, c0:c0 + 4, :], "a",
                              h1new8[:, c0:c0 + 4, :],
                              h1newd8[:, c0:c0 + 4, :])

                    # ---- layer 2 recurrent gh2 (k-half split; h2prev) ----
                    rz2 = prz2.tile([128, 2, KC, R], F32, tag="rz2", name=f"rz2_{s}")
                    n2 = pn2.tile([128, KC, R], F32, tag="n2", name=f"n2_{s}")
                    gn2 = pgn2.tile([128, KC, R], F32, tag="gn2", name=f"gn2_{s}")
                    l2sets = ((wh1s, h2p8), (wh1s, h2pd8), (wh1d, h2p8))
                    for kh in range(2):
                        for g in range(2):
                            for c in range(KC):
                                n = g * 8 + c
                                for si, (wt, ht) in enumerate(l2sets):
                                    for jp in range(2 * kh, 2 * kh + 2):
                                        nc.tensor.matmul(
                                            rz2[:, g, c, :],
                                            wt[:, 2 * jp:2 * jp + 2,
                                               n * 128:(n + 1) * 128],
                                            ht[:, 2 * jp:2 * jp + 2, :],
                                            perf_mode=DR,
                                            start=(kh == 0 and g == 0 and c == 0
                                                   and si == 0 and jp == 0),
                                            stop=False)
                        for c in range(KC):
                            n = 16 + c
                            for si, (wt, ht) in enumerate(l2sets):
                                for jp in range(2 * kh, 2 * kh + 2):
                                    nc.tensor.matmul(
                                        n2[:, c, :],
                                        wt[:, 2 * jp:2 * jp + 2,
                                           n * 128:(n + 1) * 128],
                                        ht[:, 2 * jp:2 * jp + 2, :],
                                        perf_mode=DR,
                                        start=(kh == 0 and c == 0 and si == 0
                                               and jp == 0),
                                        stop=False)
                    for g in range(2):
                        for c in range(KC):
                            n = g * 8 + c
                            nc.tensor.matmul(
                                rz2[:, g, c, :], b2rzs[0:1, n * 128:(n + 1) * 128],
                                ones, start=False, stop=False)
                    for c in range(KC):
                        nc.tensor.matmul(
                            n2[:, c, :], bhn2s[0:1, c * 128:(c + 1) * 128],
                            ones, start=False, stop=(c == 7))

                    # ---- gi2 = wi1 @ h1new, accumulated into rz2 / gn2 ----
                    # gn2's group opener is the first gi2 matmul (which waits
                    # on h1new anyway); a dep-free opener like the b2n bias
                    # matmul gets hoisted by the scheduler and then HOL-blocks
                    # the PE queue on its PSUM WAR wait.
                    gisets = ((wi1s, h1new8), (wi1s, h1newd8), (wi1d, h1new8))
                    for kh in range(2):
                        for g in range(2):
                            for c in range(KC):
                                n = g * 8 + c
                                for si, (wt, ht) in enumerate(gisets):
                                    for jp in range(2 * kh, 2 * kh + 2):
                                        nc.tensor.matmul(
                                            rz2[:, g, c, :],
                                            wt[:, 2 * jp:2 * jp + 2,
                                               n * 128:(n + 1) * 128],
                                            ht[:, 2 * jp:2 * jp + 2, :],
                                            perf_mode=DR,
                                            start=False,
                                            stop=(kh == 1 and g == 1 and c == 7
                                                  and si == 2 and jp == 3))
                        for c in range(KC):
                            n = 16 + c
                            for si, (wt, ht) in enumerate(gisets):
                                for jp in range(2 * kh, 2 * kh + 2):
                                    nc.tensor.matmul(
                                        gn2[:, c, :],
                                        wt[:, 2 * jp:2 * jp + 2,
                                           n * 128:(n + 1) * 128],
                                        ht[:, 2 * jp:2 * jp + 2, :],
                                        perf_mode=DR,
                                        start=(kh == 0 and c == 0 and si == 0
                                               and jp == 0),
                                        stop=False)
                    for c in range(KC):
                        nc.tensor.matmul(
                            gn2[:, c, :], b2ns[0:1, c * 128:(c + 1) * 128],
                            ones, start=False, stop=(c == 7))

                    if sj == 0 and bb + 1 < OBLK:
                        gnext = prefetch(bb + 1)

                    h1p8 = h1new8
                    h1pd8 = h1newd8

                    if debug:
                        nc.sync.dma_start(out=dbgh1[s], in_=h1new)
                    h1prev = h1new
                    pstate = (rz2, n2, gn2, h2new8, h2newd8, h2prev, s)
                if debug:
                    dcp = gip.tile([128, NCH, 8, R], BF16, tag="gi", name="dcp")
                    for bb in range(OBLK):
                        nc.sync.dma_start(out=dcp, in_=gi1b[bb][:])
                        nc.sync.dma_start(
                            out=dbgi[:, :, bb // 2,
                                     (bb % 2) * 8:(bb % 2) * 8 + 8, :],
                            in_=dcp)

    nc.finalize()
    return nc


def ode_traj(w1, b1, w2, b2, w3, b3):
    """RK4 trajectory of the ODE, mirroring the reference exactly (fp32)."""
    w1t = w1.T.astype(np.float32)
    w2t = w2.T.astype(np.float32)
    w3t = w3.T.astype(np.float32)

    def f(h):
        a = np.tanh(h @ w1t + b1)
        a = np.tanh(a @ w2t + b2)
        return a @ w3t + b3

    dt = np.float32((1.0 / NSEG) / SUB)
    h = np.zeros((2, H), np.float32)
    traj = []
    for _ in range(NSEG):
        for _ in range(SUB):
            k1 = f(h)
            k2 = f(h + np.float32(0.5) * dt * k1)
            k3 = f(h + np.float32(0.5) * dt * k2)
            k4 = f(h + dt * k3)
            h = h + (dt / np.float32(6.0)) * (k1 + np.float32(2.0) * k2
                                              + np.float32(2.0) * k3 + k4)
        traj.append(h.copy())
    return np.stack(traj)  # (NSEG, 2, H)


def _wtf(w):
    """[G3, H] weight -> feature-major [128, KC, G3] fp32."""
    return np.ascontiguousarray(
        w.T.reshape(KC, 128, G3).transpose(1, 0, 2)).astype(np.float32)


def _wt8(w):
    """[G3, H] weight -> feature-major [128, KC, G3] fp8, pre-scaled by SCL."""
    return np.ascontiguousarray(
        (w.T * SCL).reshape(KC, 128, G3).transpose(1, 0, 2)).astype(NP_FP8)


def _wtd8(w):
    """fp8 quantization residual of _wt8(w), itself in fp8."""
    w16 = np.ascontiguousarray(
        (w.T * SCL).reshape(KC, 128, G3).transpose(1, 0, 2)).astype(np.float32)
    return (w16 - w16.astype(NP_FP8).astype(np.float32)).astype(NP_FP8)


def make_in_maps(x, w1, b1, w2, b2, w3, b3, wi0, wh0, bi0, bh0,
                 wi1, wh1, bi1, bh1, steps=S, cores=NCORES):
    traj = ode_traj(w1, b1, w2, b2, w3, b3)

    # xt[p, bb, k, sj*R + r] = x[bb*8+sj, r, k*128+p]
    xp = np.zeros((128, KC, steps, R), np.float32)
    xp[:, :, :, :T] = x[:steps].reshape(steps, T, KC, 128).transpose(3, 2, 0, 1)
    xtf = np.ascontiguousarray(
        xp.reshape(128, KC, steps // 8, 8 * R).transpose(0, 2, 1, 3))
    xt8 = xtf.astype(NP_FP8)
    xtd8 = (xtf - xt8.astype(np.float32)).astype(NP_FP8)

    biasA = np.concatenate([bi0[:2 * H] + bh0[:2 * H], bi0[2 * H:]])
    wi0f = np.ascontiguousarray(
        (SCL * _wtf(wi0)).reshape(128, KC, 4, G3 // 4).transpose(0, 2, 1, 3))
    wi0q8 = wi0f.astype(NP_FP8)
    wi0qd8 = (wi0f - wi0q8.astype(np.float32)).astype(NP_FP8)
    shared = {
        "xt": xt8, "xtd": xtd8,
        "wi0t": wi0q8, "wi0dt": wi0qd8,
        "wh0t": _wt8(wh0), "wh0dt": _wtd8(wh0),
        "wi1t": _wt8(wi1), "wh1t": _wt8(wh1),
        "wi1dt": _wtd8(wi1), "wh1dt": _wtd8(wh1),
        "biasA": np.ascontiguousarray(
            SCL * biasA.reshape(NCH, 128).T).astype(np.float32),
        "b2rz": (SCL * (bi1[:2 * H] + bh1[:2 * H])).reshape(1, 2 * H).astype(NP_BF16),
        "b2n": (SCL * bi1[2 * H:]).reshape(1, H).astype(NP_BF16),
        "bhn1": (SCL * bh0[2 * H:]).reshape(1, H).astype(NP_BF16),
        "bhn2": (SCL * bh1[2 * H:]).reshape(1, H).astype(NP_BF16),
    }
    in_maps = []
    for i in range(cores):
        m = dict(shared)
        for nm, vec in (("h1t0", traj[i, 0]), ("h2t0", traj[i, 1])):
            ht = np.ascontiguousarray(np.broadcast_to(
                vec.reshape(KC, 128).T[:, :, None], (128, KC, R)))
            m[nm] = ht.astype(NP_BF16)
            m[nm + "8"] = ht.astype(NP_FP8)
            m[nm + "d8"] = (ht - ht.astype(NP_FP8).astype(np.float32)
                            ).astype(NP_FP8)
        in_maps.append(m)
    return in_maps


_NC_CACHE = {}


def _get_nc(steps):
    if steps not in _NC_CACHE:
        _NC_CACHE[steps] = build_nc(steps)
    return _NC_CACHE[steps]


def run_cores(inputs, steps=S, cores=NCORES, **run_kwargs):
    in_maps = make_in_maps(steps=steps, cores=cores, **inputs)
    nc = _get_nc(steps)
    return run_bass_kernel_spmd(nc, in_maps, core_ids=list(range(cores)),
                                **run_kwargs)


def kernel(x, w1, b1, w2, b2, w3, b3, wi0, wh0, bi0, bh0,
           wi1, wh1, bi1, bh1):
    args = dict(x=x, w1=w1, b1=b1, w2=w2, b2=b2, w3=w3, b3=b3,
                wi0=wi0, wh0=wh0, bi0=bi0, bh0=bh0,
                wi1=wi1, wh1=wh1, bi1=bi1, bh1=bh1)
    args = {k: np.asarray(v, np.float32) for k, v in args.items()}
    res = run_cores(args, steps=S, cores=NCORES)
    full = np.empty((S, T * NCORES, H), np.float32)
    for i in range(NCORES):
        o = np.asarray(res.results[i]["out"]).astype(np.float32)
        # [ob, p, k, sj, r] -> [s, r, feat]
        o = o.transpose(0, 3, 4, 2, 1).reshape(S, R, H)
        full[:, i::NCORES, :] = o[:, :T, :]
    return full
